# revision 10
# baseline (speedup 1.0000x reference)
"""GATv2 (nn_GATv2_59184649339075) Bass kernel for TRN2, 8-core SPMD.

Self-contained: kernel(**inputs) takes the full unsharded inputs
(x[50000,64], W[64,64], b[64], a[64], edge_index[2,800000] int32) and
returns the full [50000,64] float32 output.

Strategy (dst-partition dense layout + per-tile overflow, no collectives):
  - Host: pad nodes to 51200 (400 tiles of 128); each core owns 50
    consecutive dst tiles. For each dst node, its first J0 incoming edges
    go to a DENSE slot grid [partition = dst%128, j<J0]; the rest go to a
    small per-tile OVERFLOW region of static capacity `cap`.
  - Device per core: Wh = x@W.T + b computed on-chip in f16 (bias folded
    as an augmented contraction row) into a DRAM table stored in a
    partition-major permuted layout so the stage writes are contiguous
    1KB runs; the core's own 56 dst tiles live in SBUF for the whole run.
  - Gathers fetch 256B PAIR rows (two f16 nodes) via SWDGE with int16
    pair indices (table_row>>1 < 32768); copy_predicated in place on the
    gathered pairs selects the parity half.
  - Dense pipeline: s = Whsrc + Whdst (Whdst is a free-dim broadcast of
    the tile's SBUF-resident Wh rows - no gather, no one-hot), LeakyReLU
    on ACT, a-mul + feature tree-reduce for logits (scalar_tensor_tensor
    keeps DVE 2x/4x modes), exp on ACT, validity mask, weighted values,
    then a j tree-reduce for numerator/denominator: aggregation is pure
    free-dim reduction because partition == dst%128.
  - Overflow pipeline (~10% of edges): chunk slots with one-hot is_equal
    and PE matmul accumulation into per-tile PSUM; src AND dst rows both
    gathered as pairs.
  - Output: sigmoid((num_dense + num_ovf) / (den_dense + den_ovf)),
    written batched in a permuted layout, unpermuted on host.
"""
import sys

sys.path.insert(0, "/opt/trn_rl_repo")
from contextlib import ExitStack
from dataclasses import dataclass

import numpy as np

import concourse.bass as bass
import concourse.tile as tile
from concourse import bacc, mybir

F32 = mybir.dt.float32
F16 = mybir.dt.float16
I16 = mybir.dt.int16
I32 = mybir.dt.int32
I8 = mybir.dt.int8
AF = mybir.ActivationFunctionType
ALU = mybir.AluOpType
AX = mybir.AxisListType

N_CORES = 8
P = 128
DIN = 64
DOUT = 64
NSLOPE = 0.2
N_PAD = 51200
TILES = N_PAD // P             # 400
TILES_CORE = TILES // N_CORES  # 50
N_LOC = TILES_CORE * P         # 6400
GB = 5                         # tiles per batch
N_BATCH = TILES_CORE // GB     # 10
GT = 8                         # tiles per wh-stage matmul group
LOC_T = ((TILES_CORE + GT - 1) // GT) * GT  # 56 local tiles (padded)
N_LOC_W = LOC_T * P            # 7168


def row_of_node(n):
    """Table-row permutation: within each 1024-node group the wh stage
    writes partition-major, so node (group, g, lane) lands at row
    group*1024 + lane*8 + g. Works on arrays."""
    return (n // 1024) * 1024 + (n % P) * GT + (n % 1024) // P


@dataclass(frozen=True)
class GatCfg:
    j0: int      # dense slots per dst node
    cap: int     # overflow slots per tile (multiple of 64)

    @property
    def s_d(self):
        return GB * self.j0  # dense chunks (slots/128) per batch

    @property
    def oc(self):
        return (GB * self.cap + P - 1) // P  # overflow chunks per batch

    @property
    def s_o(self):
        return self.oc * P  # padded overflow slots per batch

    def segments(self):
        """Static per-batch overflow layout: for each tile t in the batch,
        the (chunk, lane_lo, lane_hi) pieces covering its cap slots."""
        segs = []
        for t in range(GB):
            lo, hi = t * self.cap, (t + 1) * self.cap
            cur = []
            c = lo // P
            while lo < hi:
                e = min(hi, (c + 1) * P)
                cur.append((c, lo - c * P, e - c * P))
                lo, c = e, c + 1
            segs.append(cur)
        return segs


def wrap16(idx):
    """Slot i of a gather call -> idx array position [i%16, i//16],
    replicated to the 128 partitions."""
    n = len(idx)
    assert n % 16 == 0
    a = idx.reshape(n // 16, 16).T.astype(np.int16)
    return np.tile(a, (8, 1))


def prepare(x, W, b, a, edge_index):
    N = x.shape[0]
    E = edge_index.shape[1]
    assert N <= N_PAD and N_PAD // 2 <= 32768
    src = edge_index[0].astype(np.int64)
    dst = edge_index[1].astype(np.int64)

    order = np.lexsort((src, dst))
    src_s, dst_s = src[order], dst[order]
    deg = np.bincount(dst_s, minlength=N_PAD)
    starts = np.concatenate([[0], np.cumsum(deg)[:-1]])
    rank = np.arange(E) - np.repeat(starts, deg)

    # choose j0 minimizing total gather descriptors (static across cores)
    best = None
    for j0 in range(8, 33, 2):
        ovf_t = np.maximum(deg - j0, 0).reshape(TILES, P).sum(axis=1)
        # cap % 64 == 0 keeps overflow segment base partitions in {0, 64},
        # which the PE matmul base-partition constraint requires
        cap = max(int(np.ceil(ovf_t.max() / 64) * 64), 64)
        s_o = ((GB * cap + P - 1) // P) * P
        desc = TILES_CORE * j0 * P + 2 * N_BATCH * s_o
        if best is None or desc < best[0]:
            best = (desc, j0, cap)
    _, j0, cap = best
    cfg = GatCfg(j0=j0, cap=cap)
    s_d, oc, s_o = cfg.s_d, cfg.oc, cfg.s_o

    tile_id = dst_s >> 7
    b_of = (tile_id % TILES_CORE) // GB
    t_in_b = (tile_id % TILES_CORE) % GB

    dense = rank < j0
    # dense slot within core: (batch*s_d + t_in_b*j0 + rank)*P + dst%128
    d_slot = (b_of * s_d + t_in_b * j0 + rank) * P + (dst_s & (P - 1))
    # overflow: position = cumcount within the dst TILE (edges are sorted
    # by dst so tiles are contiguous among the ovf subset)
    ovf = ~dense
    tile_ovf = tile_id[ovf]
    cnt_t = np.bincount(tile_ovf, minlength=TILES)
    assert cnt_t.max() <= cap
    st_t = np.concatenate([[0], np.cumsum(cnt_t)[:-1]])
    o_pos_tile = np.arange(int(ovf.sum())) - np.repeat(st_t, cnt_t)
    o_slot = (b_of[ovf] * s_o) + t_in_b[ovf] * cap + o_pos_tile

    rsrc = row_of_node(src_s)
    rdst = row_of_node(dst_s)

    xT = np.zeros((DIN + 1, N_PAD), np.float16)
    xT[:DIN, :N] = x.T.astype(np.float16)
    xT[DIN, :] = 1.0
    WT = np.concatenate([W.T.astype(np.float16),
                         b.reshape(1, DOUT).astype(np.float16)])
    a_row = a.reshape(1, DOUT).astype(np.float16)

    core_of = tile_id // TILES_CORE
    core_ovf = core_of[ovf]
    rsrc_o, rdst_o, dst_o = rsrc[ovf], rdst[ovf], dst_s[ovf]

    fd = s_d * P // 16  # idxD cols per batch
    fo = s_o // 16      # idxO cols per batch
    MC = s_d + 2 * oc   # merged f16 mask cols per batch
    PC = s_d + 2 * oc   # merged int8 parity cols per batch

    in_maps = []
    for c in range(N_CORES):
        m_d = dense & (core_of == c)
        m_o = core_ovf == c

        nd = N_BATCH * s_d * P
        idxD = np.zeros(nd, np.int16)
        parD = np.zeros(nd, np.int8)
        valD = np.zeros(nd, np.float16)
        sl = d_slot[m_d]
        idxD[sl] = (rsrc[m_d] >> 1).astype(np.int16)
        parD[sl] = (rsrc[m_d] & 1).astype(np.int8)
        valD[sl] = 1.0

        no = N_BATCH * s_o
        idxOS = np.zeros(no, np.int16)
        idxOD = np.zeros(no, np.int16)
        parOS = np.zeros(no, np.int8)
        parOD = np.zeros(no, np.int8)
        dtlO = np.full(no, -1.0, np.float16)
        valO = np.zeros(no, np.float16)
        so = o_slot[m_o]
        idxOS[so] = (rsrc_o[m_o] >> 1).astype(np.int16)
        idxOD[so] = (rdst_o[m_o] >> 1).astype(np.int16)
        parOS[so] = (rsrc_o[m_o] & 1).astype(np.int8)
        parOD[so] = (rdst_o[m_o] & 1).astype(np.int8)
        dtlO[so] = (dst_o[m_o] & (P - 1)).astype(np.float16)
        valO[so] = 1.0

        idxD_w = np.zeros((P, N_BATCH * fd), np.int16)
        idxOS_w = np.zeros((P, N_BATCH * fo), np.int16)
        idxOD_w = np.zeros((P, N_BATCH * fo), np.int16)
        for i in range(N_BATCH):
            idxD_w[:, i * fd:(i + 1) * fd] = wrap16(
                idxD[i * s_d * P:(i + 1) * s_d * P])
            idxOS_w[:, i * fo:(i + 1) * fo] = wrap16(
                idxOS[i * s_o:(i + 1) * s_o])
            idxOD_w[:, i * fo:(i + 1) * fo] = wrap16(
                idxOD[i * s_o:(i + 1) * s_o])

        def lane_major(v, nslots):
            # v: [nslots*P] slot = chunk*P + lane -> [P(lane), nslots]
            return v.reshape(nslots, P).T

        # merged per-batch planes:
        # f16 msk: [valD s_d | dtlO oc | valO oc]
        # int8 par: [parD s_d | parOS oc | parOD oc]
        msk = np.zeros((P, N_BATCH, MC), np.float16)
        msk[:, :, 0:s_d] = lane_major(valD, N_BATCH * s_d).reshape(P, N_BATCH, s_d)
        msk[:, :, s_d:s_d + oc] = lane_major(dtlO, N_BATCH * oc).reshape(P, N_BATCH, oc)
        msk[:, :, s_d + oc:MC] = lane_major(valO, N_BATCH * oc).reshape(P, N_BATCH, oc)
        par = np.zeros((P, N_BATCH, PC), np.int8)
        par[:, :, 0:s_d] = lane_major(parD, N_BATCH * s_d).reshape(P, N_BATCH, s_d)
        par[:, :, s_d:s_d + oc] = lane_major(parOS, N_BATCH * oc).reshape(P, N_BATCH, oc)
        par[:, :, s_d + oc:PC] = lane_major(parOD, N_BATCH * oc).reshape(P, N_BATCH, oc)

        xTs = np.zeros((DIN + 1, N_LOC_W), np.float16)
        xTs[:, :N_LOC] = xT[:, c * N_LOC:(c + 1) * N_LOC]

        in_maps.append({
            "xT": xT, "xTs": xTs, "WT": WT, "a": a_row,
            "idxD": idxD_w, "idxOS": idxOS_w, "idxOD": idxOD_w,
            "msk": np.ascontiguousarray(msk.reshape(P, N_BATCH * MC)),
            "par": np.ascontiguousarray(par.reshape(P, N_BATCH * PC)),
        })
    return cfg, in_maps, {"N": N}


def build(cfg: GatCfg, reps=1):
    nc = bacc.Bacc("TRN2", target_bir_lowering=False, debug=False,
                   num_devices=N_CORES, num_swdge_queues=4)
    j0, cap = cfg.j0, cfg.cap
    s_d, oc, s_o = cfg.s_d, cfg.oc, cfg.s_o
    fd = s_d * P // 16
    fo = s_o // 16
    MC = s_d + 2 * oc
    PC = s_d + 2 * oc
    segs = cfg.segments()

    xT_d = nc.dram_tensor("xT", [DIN + 1, N_PAD], F16, kind="ExternalInput").ap()
    xTs_d = nc.dram_tensor("xTs", [DIN + 1, N_LOC_W], F16, kind="ExternalInput").ap()
    WT_d = nc.dram_tensor("WT", [DIN + 1, DOUT], F16, kind="ExternalInput").ap()
    a_d = nc.dram_tensor("a", [1, DOUT], F16, kind="ExternalInput").ap()
    idxD_d = nc.dram_tensor("idxD", [P, N_BATCH * fd], I16, kind="ExternalInput").ap()
    idxOS_d = nc.dram_tensor("idxOS", [P, N_BATCH * fo], I16, kind="ExternalInput").ap()
    idxOD_d = nc.dram_tensor("idxOD", [P, N_BATCH * fo], I16, kind="ExternalInput").ap()
    msk_d = nc.dram_tensor("msk", [P, N_BATCH * MC], F16, kind="ExternalInput").ap()
    par_d = nc.dram_tensor("par", [P, N_BATCH * PC], I8, kind="ExternalInput").ap()
    out_d = nc.dram_tensor("out", [N_LOC, DOUT], F32, kind="ExternalOutput").ap()
    wh_d = nc.dram_tensor("wh", [N_PAD, DOUT], F16).ap()
    # pair view for gathers: row q = table rows (2q, 2q+1), 256B
    whp = wh_d.rearrange("(q two) f -> q (two f)", two=2)

    with tile.TileContext(nc) as tc:
        with ExitStack() as ctx:
            cpool = ctx.enter_context(tc.tile_pool(name="const", bufs=1))
            WT_sb = cpool.tile([DIN + 1, DOUT], F16)
            nc.sync.dma_start(WT_sb[:], WT_d[:, :])
            a_rep = cpool.tile([P, DOUT], F16)
            nc.sync.dma_start(a_rep[:], a_d.to_broadcast((P, DOUT)))
            iota_i = cpool.tile([P, P], I32)
            nc.gpsimd.iota(iota_i[:], pattern=[[1, P]], base=0, channel_multiplier=0)
            iota_f = cpool.tile([P, P], F16)
            nc.vector.tensor_copy(iota_f[:], iota_i[:])
            # the core's own dst-tile Wh rows, resident for the whole run
            whd_all = cpool.tile([P, LOC_T, DOUT], F16)

            # wh table: permuted rows so each partition writes one
            # contiguous 1KB run per group
            with ExitStack() as c2:
                xp = c2.enter_context(tc.tile_pool(name="xt", bufs=3))
                pp = c2.enter_context(tc.tile_pool(name="whps", bufs=4, space="PSUM"))
                wp = c2.enter_context(tc.tile_pool(name="whsb", bufs=3))
                for g in range(TILES // GT):
                    t0 = g * GT
                    xt = xp.tile([DIN + 1, GT * P], F16, tag="xt")
                    nc.sync.dma_start(xt[:], xT_d[:, t0 * P:(t0 + GT) * P])
                    ps = pp.tile([P, GT, DOUT], F32, tag="ps")
                    for j in range(GT):
                        nc.tensor.matmul(ps[:, j, :], lhsT=xt[:, j * P:(j + 1) * P],
                                         rhs=WT_sb[:], start=True, stop=True)
                    whb = wp.tile([P, GT, DOUT], F16, tag="whb")
                    nc.scalar.copy(whb[:], ps[:])
                    nc.sync.dma_start(
                        wh_d[g * GT * P:(g + 1) * GT * P, :]
                        .rearrange("(p g) f -> p g f", p=P), whb[:])
                # local dst tiles -> SBUF-resident whd_all
                for g in range(LOC_T // GT):
                    t0 = g * GT
                    xt = xp.tile([DIN + 1, GT * P], F16, tag="xt")
                    nc.sync.dma_start(xt[:], xTs_d[:, t0 * P:(t0 + GT) * P])
                    ps = pp.tile([P, GT, DOUT], F32, tag="ps")
                    for j in range(GT):
                        nc.tensor.matmul(ps[:, j, :], lhsT=xt[:, j * P:(j + 1) * P],
                                         rhs=WT_sb[:], start=True, stop=True)
                    nc.scalar.copy(whd_all[:, t0:t0 + GT, :], ps[:])

            ipool = ctx.enter_context(tc.tile_pool(name="idx", bufs=2))
            mpool = ctx.enter_context(tc.tile_pool(name="mask", bufs=2))
            gpool = ctx.enter_context(tc.tile_pool(name="gath", bufs=2))
            wpool = ctx.enter_context(tc.tile_pool(name="work", bufs=2))
            apool = ctx.enter_context(tc.tile_pool(name="aggp", bufs=4, space="PSUM"))
            npool = ctx.enter_context(tc.tile_pool(name="outp", bufs=2))

            def stt(out, in0, op0, scalar, op1, in1):
                nc.vector.scalar_tensor_tensor(
                    out, in0, scalar, in1, op0=op0, op1=op1)

            def score(u, e32, e16, ef32, n):
                """u [P,n,64] f16 -> leaky-relu, a-mul (in place),
                tree-reduce features into ef32 [P,n] f32."""
                stt(u[:], u[:], ALU.mult, NSLOPE, ALU.max, u[:])
                stt(u[:], u[:], ALU.add, 0.0, ALU.mult,
                    a_rep[:].unsqueeze(1).to_broadcast((P, n, DOUT)))
                stt(e32[:], u[:, :, 0:32], ALU.add, 0.0, ALU.add, u[:, :, 32:64])
                stt(e16[:], e32[:, :, 0:16], ALU.add, 0.0, ALU.add,
                    e32[:, :, 16:32])
                stt(e32[:, :, 0:8], e16[:, :, 0:8], ALU.add, 0.0, ALU.add,
                    e16[:, :, 8:16])
                stt(e16[:, :, 0:4], e32[:, :, 0:4], ALU.add, 0.0, ALU.add,
                    e32[:, :, 4:8])
                stt(e32[:, :, 0:2], e16[:, :, 0:2], ALU.add, 0.0, ALU.add,
                    e16[:, :, 2:4])
                stt(ef32.unsqueeze(2), e32[:, :, 0:1], ALU.add, 0.0, ALU.add,
                    e32[:, :, 1:2])

            half1 = (j0 + 1) // 2

            for _ in range(reps):
                for bi in range(N_BATCH):
                    idxD_t = ipool.tile([P, fd], I16, tag="id")
                    nc.sync.dma_start(idxD_t[:], idxD_d[:, bi * fd:(bi + 1) * fd])
                    idxOS_t = ipool.tile([P, fo], I16, tag="ios")
                    nc.sync.dma_start(idxOS_t[:], idxOS_d[:, bi * fo:(bi + 1) * fo])
                    idxOD_t = ipool.tile([P, fo], I16, tag="iod")
                    nc.sync.dma_start(idxOD_t[:], idxOD_d[:, bi * fo:(bi + 1) * fo])
                    msk_t = mpool.tile([P, MC], F16, tag="msk")
                    nc.sync.dma_start(msk_t[:], msk_d[:, bi * MC:(bi + 1) * MC])
                    par_t = mpool.tile([P, PC], I8, tag="par")
                    nc.sync.dma_start(par_t[:], par_d[:, bi * PC:(bi + 1) * PC])
                    valD_v = msk_t[:, 0:s_d]
                    dtlO_v = msk_t[:, s_d:s_d + oc]
                    valO_v = msk_t[:, s_d + oc:MC]
                    parD_v = par_t[:, 0:s_d]
                    parOS_v = par_t[:, s_d:s_d + oc]
                    parOD_v = par_t[:, s_d + oc:PC]

                    pairs = gpool.tile([P, s_d, P], F16, tag="pairs")
                    nc.gpsimd.dma_gather(
                        out_ap=pairs[:], in_ap=whp[:, :], idxs_ap=idxD_t[:],
                        num_idxs=s_d * P, num_idxs_reg=s_d * P,
                        elem_size=P, single_packet=False, queue_num=0)
                    pairsOS = gpool.tile([P, oc, P], F16, tag="pairsOS")
                    nc.gpsimd.dma_gather(
                        out_ap=pairsOS[:], in_ap=whp[:, :], idxs_ap=idxOS_t[:],
                        num_idxs=s_o, num_idxs_reg=s_o,
                        elem_size=P, single_packet=False, queue_num=0)
                    pairsOD = gpool.tile([P, oc, P], F16, tag="pairsOD")
                    nc.gpsimd.dma_gather(
                        out_ap=pairsOD[:], in_ap=whp[:, :], idxs_ap=idxOD_t[:],
                        num_idxs=s_o, num_idxs_reg=s_o,
                        elem_size=P, single_packet=False, queue_num=0)

                    # ---------------- dense pipeline ----------------
                    nc.vector.copy_predicated(
                        pairs[:, :, 0:DOUT],
                        parD_v.unsqueeze(2).to_broadcast((P, s_d, DOUT)),
                        pairs[:, :, DOUT:2 * DOUT])
                    whsrc = pairs[:, :, 0:DOUT]
                    u = wpool.tile([P, s_d, DOUT], F16, tag="u")
                    for t in range(GB):
                        stt(u[:, t * j0:(t + 1) * j0, :],
                            whsrc[:, t * j0:(t + 1) * j0, :], ALU.add, 0.0,
                            ALU.add,
                            whd_all[:, bi * GB + t, :].unsqueeze(1)
                            .to_broadcast((P, j0, DOUT)))
                    e32 = wpool.tile([P, s_d, 32], F16, tag="e32")
                    e16 = wpool.tile([P, s_d, 16], F16, tag="e16")
                    ef = wpool.tile([P, s_d], F32, tag="ef")
                    score(u, e32, e16, ef[:], s_d)
                    ex = wpool.tile([P, s_d], F32, tag="ex")
                    nc.scalar.activation(ex[:], ef[:], AF.Exp)
                    exm = wpool.tile([P, s_d], F16, tag="exm")
                    nc.vector.tensor_tensor(exm[:], ex[:], valD_v, op=ALU.mult)
                    v = wpool.tile([P, s_d, DOUT], F16, tag="v")
                    nc.vector.tensor_tensor(
                        v[:], whsrc,
                        exm[:].unsqueeze(2).to_broadcast((P, s_d, DOUT)),
                        op=ALU.mult)
                    # j-tree reduce num: [P, GB, j0, 64] -> [P, GB, 64]
                    tA = wpool.tile([P, GB, half1, DOUT], F16, tag="tA")
                    tB = wpool.tile([P, GB, max(half1 // 2, 1), DOUT], F16,
                                    tag="tB")
                    cur = v[:].rearrange("p (t j) f -> p t j f", t=GB)
                    cn = j0
                    buf = [tA, tB]
                    ti = 0
                    while cn > 1:
                        m = cn // 2
                        nxt = buf[ti][:]
                        ti ^= 1
                        stt(nxt[:, :, 0:m, :], cur[:, :, 0:m, :], ALU.add, 0.0,
                            ALU.add, cur[:, :, m:2 * m, :])
                        if cn % 2:
                            stt(nxt[:, :, 0:1, :], nxt[:, :, 0:1, :], ALU.add,
                                0.0, ALU.add, cur[:, :, 2 * m:2 * m + 1, :])
                        cur, cn = nxt, m
                    numd = cur  # [P, GB, >=1, DOUT], slot 0 holds the sum
                    dend = wpool.tile([P, GB], F32, tag="dend")
                    nc.vector.tensor_reduce(
                        dend[:], exm[:].rearrange("p (t j) -> p t j", t=GB),
                        axis=AX.X, op=ALU.add)

                    # ---------------- overflow pipeline ----------------
                    nc.vector.copy_predicated(
                        pairsOS[:, :, 0:DOUT],
                        parOS_v.unsqueeze(2).to_broadcast((P, oc, DOUT)),
                        pairsOS[:, :, DOUT:2 * DOUT])
                    whsO = pairsOS[:, :, 0:DOUT]
                    nc.vector.copy_predicated(
                        pairsOD[:, :, 0:DOUT],
                        parOD_v.unsqueeze(2).to_broadcast((P, oc, DOUT)),
                        pairsOD[:, :, DOUT:2 * DOUT])
                    uo = wpool.tile([P, oc, DOUT], F16, tag="uo")
                    stt(uo[:], pairsOD[:, :, 0:DOUT], ALU.add, 0.0, ALU.add,
                        whsO)
                    eo32 = wpool.tile([P, oc, 32], F16, tag="eo32")
                    eo16 = wpool.tile([P, oc, 16], F16, tag="eo16")
                    efo = wpool.tile([P, oc], F32, tag="efo")
                    score(uo, eo32, eo16, efo[:], oc)
                    exo = wpool.tile([P, oc], F32, tag="exo")
                    nc.scalar.activation(exo[:], efo[:], AF.Exp)
                    exmo = wpool.tile([P, oc], F16, tag="exmo")
                    nc.vector.tensor_tensor(exmo[:], exo[:], valO_v, op=ALU.mult)
                    vo = wpool.tile([P, oc, DOUT + 1], F16, tag="vo")
                    nc.vector.tensor_tensor(
                        vo[:, :, 0:DOUT], whsO,
                        exmo[:].unsqueeze(2).to_broadcast((P, oc, DOUT)),
                        op=ALU.mult)
                    nc.vector.tensor_copy(vo[:, :, DOUT:DOUT + 1],
                                          exmo[:].unsqueeze(2))
                    oh = wpool.tile([P, oc, P], F16, tag="oh")
                    nc.vector.tensor_tensor(
                        oh[:],
                        dtlO_v.unsqueeze(2).to_broadcast((P, oc, P)),
                        iota_f[:].unsqueeze(1).to_broadcast((P, oc, P)),
                        op=ALU.is_equal)

                    # ---------------- combine + output ----------------
                    obb = npool.tile([P, GB, DOUT], F32, tag="obb")
                    for t in range(GB):
                        ps = apool.tile([P, DOUT + 1], F32, tag="agg")
                        seg_list = segs[t]
                        for k, (c, l0, l1) in enumerate(seg_list):
                            nc.tensor.matmul(
                                ps[:], lhsT=oh[l0:l1, c, :], rhs=vo[l0:l1, c, :],
                                start=(k == 0), stop=(k == len(seg_list) - 1))
                        numf = npool.tile([P, DOUT], F32, tag="numf")
                        stt(numf[:], numd[:, t, 0, :], ALU.add, 0.0, ALU.add,
                            ps[:, 0:DOUT])
                        denf = npool.tile([P, 1], F32, tag="denf")
                        stt(denf[:], dend[:, t:t + 1], ALU.add, 0.0,
                            ALU.add, ps[:, DOUT:DOUT + 1])
                        nc.vector.tensor_scalar_max(denf[:], denf[:], 1e-9)
                        rec = npool.tile([P, 1], F32, tag="rec")
                        nc.vector.reciprocal(rec[:], denf[:])
                        nc.scalar.activation(obb[:, t, :], numf[:], AF.Sigmoid,
                                             scale=rec[:])
                    # permuted batched output write: row = bi*640 + p*GB + t
                    nc.sync.dma_start(
                        out_d[bi * GB * P:(bi + 1) * GB * P, :]
                        .rearrange("(p t) f -> p t f", p=P), obb[:])

    nc.compile()
    return nc


_CACHE = {}


def unpermute_out(arr):
    """Inverse of the permuted batched output write."""
    return arr.reshape(N_BATCH, P, GB, DOUT).transpose(0, 2, 1, 3).reshape(
        N_LOC, DOUT)


def kernel(x, W, b, a, edge_index):
    x = np.ascontiguousarray(np.asarray(x, dtype=np.float32))
    W = np.ascontiguousarray(np.asarray(W, dtype=np.float32))
    b = np.ascontiguousarray(np.asarray(b, dtype=np.float32))
    a = np.ascontiguousarray(np.asarray(a, dtype=np.float32))
    edge_index = np.asarray(edge_index)

    cfg, in_maps, meta = prepare(x, W, b, a, edge_index)
    nc = _CACHE.get(cfg)
    if nc is None:
        nc = build(cfg)
        _CACHE[cfg] = nc

    from concourse.bass_utils import run_bass_kernel_spmd
    res = run_bass_kernel_spmd(nc, in_maps, core_ids=list(range(N_CORES)))
    parts = [unpermute_out(res.results[c]["out"]) for c in range(N_CORES)]
    return np.concatenate(parts, axis=0)[:meta["N"]].astype(np.float32)


# revision 11
# speedup vs baseline: 1.5580x; 1.5580x over previous
"""GATv2 (nn_GATv2_59184649339075) Bass kernel for TRN2, 8-core SPMD.

Self-contained: kernel(**inputs) takes the full unsharded inputs
(x[50000,64], W[64,64], b[64], a[64], edge_index[2,800000] int32) and
returns the full [50000,64] float32 output.

Strategy (dst-partition dense layout + per-tile overflow, no collectives):
  - Host: pad nodes to 51200 (400 tiles of 128); each core owns 50
    consecutive dst tiles. For each dst node, its first J0 incoming edges
    go to a DENSE slot grid [partition = dst%128, j<J0]; the rest go to a
    small per-tile OVERFLOW region of static capacity `cap`.
  - Device per core: Wh = x@W.T + b computed on-chip in f16 (bias folded
    as an augmented contraction row) into a DRAM table stored in a
    partition-major permuted layout so the stage writes are contiguous
    1KB runs; the core's own 56 dst tiles live in SBUF for the whole run.
  - Gathers fetch 256B PAIR rows (two f16 nodes) via SWDGE with int16
    pair indices (table_row>>1 < 32768); copy_predicated in place on the
    gathered pairs selects the parity half.
  - Dense pipeline: s = Whsrc + Whdst (Whdst is a free-dim broadcast of
    the tile's SBUF-resident Wh rows - no gather, no one-hot), LeakyReLU
    on ACT, a-mul + feature tree-reduce for logits (scalar_tensor_tensor
    keeps DVE 2x/4x modes), exp on ACT, validity mask, weighted values,
    then a j tree-reduce for numerator/denominator: aggregation is pure
    free-dim reduction because partition == dst%128.
  - Overflow pipeline (~10% of edges): chunk slots with one-hot is_equal
    and PE matmul accumulation into per-tile PSUM; src AND dst rows both
    gathered as pairs.
  - Output: sigmoid((num_dense + num_ovf) / (den_dense + den_ovf)),
    written batched in a permuted layout, unpermuted on host.
"""
import sys

sys.path.insert(0, "/opt/trn_rl_repo")
from contextlib import ExitStack
from dataclasses import dataclass

import numpy as np

import concourse.bass as bass
import concourse.tile as tile
from concourse import bacc, mybir

F32 = mybir.dt.float32
F16 = mybir.dt.float16
I16 = mybir.dt.int16
I32 = mybir.dt.int32
I8 = mybir.dt.int8
AF = mybir.ActivationFunctionType
ALU = mybir.AluOpType
AX = mybir.AxisListType

N_CORES = 8
P = 128
DIN = 64
DOUT = 64
NSLOPE = 0.2
N_PAD = 51200
TILES = N_PAD // P             # 400
TILES_CORE = TILES // N_CORES  # 50
N_LOC = TILES_CORE * P         # 6400
GB = 5                         # tiles per batch
N_BATCH = TILES_CORE // GB     # 10
GT = 8                         # tiles per wh-stage matmul group
LOC_T = ((TILES_CORE + GT - 1) // GT) * GT  # 56 local tiles (padded)
N_LOC_W = LOC_T * P            # 7168


def row_of_node(n):
    """Table-row permutation: within each 1024-node group the wh stage
    writes partition-major, so node (group, g, lane) lands at row
    group*1024 + lane*8 + g. Works on arrays."""
    return (n // 1024) * 1024 + (n % P) * GT + (n % 1024) // P


@dataclass(frozen=True)
class GatCfg:
    j0: int      # dense slots per dst node
    cap: int     # overflow slots per tile (multiple of 64)

    @property
    def s_d(self):
        return GB * self.j0  # dense chunks (slots/128) per batch

    @property
    def oc(self):
        return (GB * self.cap + P - 1) // P  # overflow chunks per batch

    @property
    def s_o(self):
        return self.oc * P  # padded overflow slots per batch

    def segments(self):
        """Static per-batch overflow layout: for each tile t in the batch,
        the (chunk, lane_lo, lane_hi) pieces covering its cap slots."""
        segs = []
        for t in range(GB):
            lo, hi = t * self.cap, (t + 1) * self.cap
            cur = []
            c = lo // P
            while lo < hi:
                e = min(hi, (c + 1) * P)
                cur.append((c, lo - c * P, e - c * P))
                lo, c = e, c + 1
            segs.append(cur)
        return segs


def wrap16(idx):
    """Slot i of a gather call -> idx array position [i%16, i//16],
    replicated to the 128 partitions."""
    n = len(idx)
    assert n % 16 == 0
    a = idx.reshape(n // 16, 16).T.astype(np.int16)
    return np.tile(a, (8, 1))


def prepare(x, W, b, a, edge_index):
    N = x.shape[0]
    E = edge_index.shape[1]
    assert N <= N_PAD and N_PAD // 2 <= 32768
    src = edge_index[0].astype(np.int64)
    dst = edge_index[1].astype(np.int64)

    order = np.lexsort((src, dst))
    src_s, dst_s = src[order], dst[order]
    deg = np.bincount(dst_s, minlength=N_PAD)
    starts = np.concatenate([[0], np.cumsum(deg)[:-1]])
    rank = np.arange(E) - np.repeat(starts, deg)

    # choose j0 minimizing total gather descriptors (static across cores)
    best = None
    for j0 in range(8, 33, 2):
        ovf_t = np.maximum(deg - j0, 0).reshape(TILES, P).sum(axis=1)
        # cap % 64 == 0 keeps overflow segment base partitions in {0, 64},
        # which the PE matmul base-partition constraint requires
        cap = max(int(np.ceil(ovf_t.max() / 64) * 64), 64)
        s_o = ((GB * cap + P - 1) // P) * P
        desc = TILES_CORE * j0 * P + 2 * N_BATCH * s_o
        if best is None or desc < best[0]:
            best = (desc, j0, cap)
    _, j0, cap = best
    cfg = GatCfg(j0=j0, cap=cap)
    s_d, oc, s_o = cfg.s_d, cfg.oc, cfg.s_o

    tile_id = dst_s >> 7
    b_of = (tile_id % TILES_CORE) // GB
    t_in_b = (tile_id % TILES_CORE) % GB

    dense = rank < j0
    # dense slot within core: (batch*s_d + t_in_b*j0 + rank)*P + dst%128
    d_slot = (b_of * s_d + t_in_b * j0 + rank) * P + (dst_s & (P - 1))
    # overflow: position = cumcount within the dst TILE (edges are sorted
    # by dst so tiles are contiguous among the ovf subset)
    ovf = ~dense
    tile_ovf = tile_id[ovf]
    cnt_t = np.bincount(tile_ovf, minlength=TILES)
    assert cnt_t.max() <= cap
    st_t = np.concatenate([[0], np.cumsum(cnt_t)[:-1]])
    o_pos_tile = np.arange(int(ovf.sum())) - np.repeat(st_t, cnt_t)
    o_slot = (b_of[ovf] * s_o) + t_in_b[ovf] * cap + o_pos_tile

    rsrc = row_of_node(src_s)
    rdst = row_of_node(dst_s)

    xT = np.zeros((DIN + 1, N_PAD), np.float16)
    xT[:DIN, :N] = x.T.astype(np.float16)
    xT[DIN, :] = 1.0
    WT = np.concatenate([W.T.astype(np.float16),
                         b.reshape(1, DOUT).astype(np.float16)])
    a_row = a.reshape(1, DOUT).astype(np.float16)

    core_of = tile_id // TILES_CORE
    core_ovf = core_of[ovf]
    rsrc_o, rdst_o, dst_o = rsrc[ovf], rdst[ovf], dst_s[ovf]

    fd = s_d * P // 16  # idxD cols per batch
    fo = s_o // 16      # idxO cols per batch
    MC = s_d + 2 * oc   # merged f16 mask cols per batch
    PC = s_d + 2 * oc   # merged int8 parity cols per batch

    in_maps = []
    for c in range(N_CORES):
        m_d = dense & (core_of == c)
        m_o = core_ovf == c

        nd = N_BATCH * s_d * P
        idxD = np.zeros(nd, np.int16)
        parD = np.zeros(nd, np.int8)
        valD = np.zeros(nd, np.float16)
        sl = d_slot[m_d]
        idxD[sl] = (rsrc[m_d] >> 1).astype(np.int16)
        parD[sl] = (rsrc[m_d] & 1).astype(np.int8)
        valD[sl] = 1.0

        no = N_BATCH * s_o
        idxOS = np.zeros(no, np.int16)
        idxOD = np.zeros(no, np.int16)
        parOS = np.zeros(no, np.int8)
        parOD = np.zeros(no, np.int8)
        dtlO = np.full(no, -1.0, np.float16)
        valO = np.zeros(no, np.float16)
        so = o_slot[m_o]
        idxOS[so] = (rsrc_o[m_o] >> 1).astype(np.int16)
        idxOD[so] = (rdst_o[m_o] >> 1).astype(np.int16)
        parOS[so] = (rsrc_o[m_o] & 1).astype(np.int8)
        parOD[so] = (rdst_o[m_o] & 1).astype(np.int8)
        dtlO[so] = (dst_o[m_o] & (P - 1)).astype(np.float16)
        valO[so] = 1.0

        idxD_w = np.zeros((P, N_BATCH * fd), np.int16)
        idxOS_w = np.zeros((P, N_BATCH * fo), np.int16)
        idxOD_w = np.zeros((P, N_BATCH * fo), np.int16)
        for i in range(N_BATCH):
            idxD_w[:, i * fd:(i + 1) * fd] = wrap16(
                idxD[i * s_d * P:(i + 1) * s_d * P])
            idxOS_w[:, i * fo:(i + 1) * fo] = wrap16(
                idxOS[i * s_o:(i + 1) * s_o])
            idxOD_w[:, i * fo:(i + 1) * fo] = wrap16(
                idxOD[i * s_o:(i + 1) * s_o])

        def lane_major(v, nslots):
            # v: [nslots*P] slot = chunk*P + lane -> [P(lane), nslots]
            return v.reshape(nslots, P).T

        # merged per-batch planes:
        # f16 msk: [valD s_d | dtlO oc | valO oc]
        # int8 par: [parD s_d | parOS oc | parOD oc]
        msk = np.zeros((P, N_BATCH, MC), np.float16)
        msk[:, :, 0:s_d] = lane_major(valD, N_BATCH * s_d).reshape(P, N_BATCH, s_d)
        msk[:, :, s_d:s_d + oc] = lane_major(dtlO, N_BATCH * oc).reshape(P, N_BATCH, oc)
        msk[:, :, s_d + oc:MC] = lane_major(valO, N_BATCH * oc).reshape(P, N_BATCH, oc)
        par = np.zeros((P, N_BATCH, PC), np.int8)
        par[:, :, 0:s_d] = lane_major(parD, N_BATCH * s_d).reshape(P, N_BATCH, s_d)
        par[:, :, s_d:s_d + oc] = lane_major(parOS, N_BATCH * oc).reshape(P, N_BATCH, oc)
        par[:, :, s_d + oc:PC] = lane_major(parOD, N_BATCH * oc).reshape(P, N_BATCH, oc)

        xTs = np.zeros((DIN + 1, N_LOC_W), np.float16)
        xTs[:, :N_LOC] = xT[:, c * N_LOC:(c + 1) * N_LOC]

        in_maps.append({
            "xT": xT, "xTs": xTs, "WT": WT, "a": a_row,
            "idxD": idxD_w, "idxOS": idxOS_w, "idxOD": idxOD_w,
            "msk": np.ascontiguousarray(msk.reshape(P, N_BATCH * MC)),
            "par": np.ascontiguousarray(par.reshape(P, N_BATCH * PC)),
        })
    return cfg, in_maps, {"N": N}


def build(cfg: GatCfg, reps=1, one_queue=False):
    nc = bacc.Bacc("TRN2", target_bir_lowering=False, debug=False,
                   num_devices=N_CORES, num_swdge_queues=4)
    j0, cap = cfg.j0, cfg.cap
    s_d, oc, s_o = cfg.s_d, cfg.oc, cfg.s_o
    fd = s_d * P // 16
    fo = s_o // 16
    MC = s_d + 2 * oc
    PC = s_d + 2 * oc
    segs = cfg.segments()

    xT_d = nc.dram_tensor("xT", [DIN + 1, N_PAD], F16, kind="ExternalInput").ap()
    xTs_d = nc.dram_tensor("xTs", [DIN + 1, N_LOC_W], F16, kind="ExternalInput").ap()
    WT_d = nc.dram_tensor("WT", [DIN + 1, DOUT], F16, kind="ExternalInput").ap()
    a_d = nc.dram_tensor("a", [1, DOUT], F16, kind="ExternalInput").ap()
    idxD_d = nc.dram_tensor("idxD", [P, N_BATCH * fd], I16, kind="ExternalInput").ap()
    idxOS_d = nc.dram_tensor("idxOS", [P, N_BATCH * fo], I16, kind="ExternalInput").ap()
    idxOD_d = nc.dram_tensor("idxOD", [P, N_BATCH * fo], I16, kind="ExternalInput").ap()
    msk_d = nc.dram_tensor("msk", [P, N_BATCH * MC], F16, kind="ExternalInput").ap()
    par_d = nc.dram_tensor("par", [P, N_BATCH * PC], I8, kind="ExternalInput").ap()
    out_d = nc.dram_tensor("out", [N_LOC, DOUT], F32, kind="ExternalOutput").ap()
    wh_d = nc.dram_tensor("wh", [N_PAD, DOUT], F16).ap()
    # pair view for gathers: row q = table rows (2q, 2q+1), 256B
    whp = wh_d.rearrange("(q two) f -> q (two f)", two=2)

    with tile.TileContext(nc) as tc:
        with ExitStack() as ctx:
            cpool = ctx.enter_context(tc.tile_pool(name="const", bufs=1))
            WT_sb = cpool.tile([DIN + 1, DOUT], F16)
            nc.sync.dma_start(WT_sb[:], WT_d[:, :])
            a_rep = cpool.tile([P, DOUT], F16)
            nc.sync.dma_start(a_rep[:], a_d.to_broadcast((P, DOUT)))
            iota_i = cpool.tile([P, P], I32)
            nc.gpsimd.iota(iota_i[:], pattern=[[1, P]], base=0, channel_multiplier=0)
            iota_f = cpool.tile([P, P], F16)
            nc.vector.tensor_copy(iota_f[:], iota_i[:])
            # the core's own dst-tile Wh rows, resident for the whole run
            whd_all = cpool.tile([P, LOC_T, DOUT], F16)

            # wh table: permuted rows so each partition writes one
            # contiguous 1KB run per group
            with ExitStack() as c2:
                xp = c2.enter_context(tc.tile_pool(name="xt", bufs=3))
                pp = c2.enter_context(tc.tile_pool(name="whps", bufs=4, space="PSUM"))
                wp = c2.enter_context(tc.tile_pool(name="whsb", bufs=3))
                for g in range(TILES // GT):
                    t0 = g * GT
                    xt = xp.tile([DIN + 1, GT * P], F16, tag="xt")
                    nc.sync.dma_start(xt[:], xT_d[:, t0 * P:(t0 + GT) * P])
                    ps = pp.tile([P, GT, DOUT], F32, tag="ps")
                    for j in range(GT):
                        nc.tensor.matmul(ps[:, j, :], lhsT=xt[:, j * P:(j + 1) * P],
                                         rhs=WT_sb[:], start=True, stop=True)
                    whb = wp.tile([P, GT, DOUT], F16, tag="whb")
                    nc.scalar.copy(whb[:], ps[:])
                    nc.sync.dma_start(
                        wh_d[g * GT * P:(g + 1) * GT * P, :]
                        .rearrange("(p g) f -> p g f", p=P), whb[:])
                # local dst tiles -> SBUF-resident whd_all
                for g in range(LOC_T // GT):
                    t0 = g * GT
                    xt = xp.tile([DIN + 1, GT * P], F16, tag="xt")
                    nc.sync.dma_start(xt[:], xTs_d[:, t0 * P:(t0 + GT) * P])
                    ps = pp.tile([P, GT, DOUT], F32, tag="ps")
                    for j in range(GT):
                        nc.tensor.matmul(ps[:, j, :], lhsT=xt[:, j * P:(j + 1) * P],
                                         rhs=WT_sb[:], start=True, stop=True)
                    nc.scalar.copy(whd_all[:, t0:t0 + GT, :], ps[:])

            ipool = ctx.enter_context(tc.tile_pool(name="idx", bufs=2))
            mpool = ctx.enter_context(tc.tile_pool(name="mask", bufs=2))
            gpool = ctx.enter_context(tc.tile_pool(name="gath", bufs=2))
            wpool = ctx.enter_context(tc.tile_pool(name="work", bufs=2))
            apool = ctx.enter_context(tc.tile_pool(name="aggp", bufs=4, space="PSUM"))
            npool = ctx.enter_context(tc.tile_pool(name="outp", bufs=2))

            def stt(out, in0, op0, scalar, op1, in1):
                nc.vector.scalar_tensor_tensor(
                    out, in0, scalar, in1, op0=op0, op1=op1)

            def score(u, e32, e16, ef32, n):
                """u [P,n,64] f16 -> leaky-relu, a-mul (in place),
                tree-reduce features into ef32 [P,n] f32."""
                stt(u[:], u[:], ALU.mult, NSLOPE, ALU.max, u[:])
                stt(u[:], u[:], ALU.add, 0.0, ALU.mult,
                    a_rep[:].unsqueeze(1).to_broadcast((P, n, DOUT)))
                stt(e32[:], u[:, :, 0:32], ALU.add, 0.0, ALU.add, u[:, :, 32:64])
                stt(e16[:], e32[:, :, 0:16], ALU.add, 0.0, ALU.add,
                    e32[:, :, 16:32])
                stt(e32[:, :, 0:8], e16[:, :, 0:8], ALU.add, 0.0, ALU.add,
                    e16[:, :, 8:16])
                stt(e16[:, :, 0:4], e32[:, :, 0:4], ALU.add, 0.0, ALU.add,
                    e32[:, :, 4:8])
                stt(e32[:, :, 0:2], e16[:, :, 0:2], ALU.add, 0.0, ALU.add,
                    e16[:, :, 2:4])
                stt(ef32.unsqueeze(2), e32[:, :, 0:1], ALU.add, 0.0, ALU.add,
                    e32[:, :, 1:2])

            half1 = (j0 + 1) // 2

            for _ in range(reps):
                for bi in range(N_BATCH):
                    q0 = 0 if one_queue else (3 * bi) % 4
                    q1 = 0 if one_queue else (q0 + 1) % 4
                    q2 = 0 if one_queue else (q0 + 2) % 4
                    idxD_t = ipool.tile([P, fd], I16, tag="id")
                    nc.sync.dma_start(idxD_t[:], idxD_d[:, bi * fd:(bi + 1) * fd])
                    idxOS_t = ipool.tile([P, fo], I16, tag="ios")
                    nc.sync.dma_start(idxOS_t[:], idxOS_d[:, bi * fo:(bi + 1) * fo])
                    idxOD_t = ipool.tile([P, fo], I16, tag="iod")
                    nc.sync.dma_start(idxOD_t[:], idxOD_d[:, bi * fo:(bi + 1) * fo])
                    msk_t = mpool.tile([P, MC], F16, tag="msk")
                    nc.sync.dma_start(msk_t[:], msk_d[:, bi * MC:(bi + 1) * MC])
                    par_t = mpool.tile([P, PC], I8, tag="par")
                    nc.sync.dma_start(par_t[:], par_d[:, bi * PC:(bi + 1) * PC])
                    valD_v = msk_t[:, 0:s_d]
                    dtlO_v = msk_t[:, s_d:s_d + oc]
                    valO_v = msk_t[:, s_d + oc:MC]
                    parD_v = par_t[:, 0:s_d]
                    parOS_v = par_t[:, s_d:s_d + oc]
                    parOD_v = par_t[:, s_d + oc:PC]

                    pairs = gpool.tile([P, s_d, P], F16, tag="pairs")
                    nc.gpsimd.dma_gather(
                        out_ap=pairs[:], in_ap=whp[:, :], idxs_ap=idxD_t[:],
                        num_idxs=s_d * P, num_idxs_reg=s_d * P,
                        elem_size=P, single_packet=False, queue_num=q0)
                    pairsOS = gpool.tile([P, oc, P], F16, tag="pairsOS")
                    nc.gpsimd.dma_gather(
                        out_ap=pairsOS[:], in_ap=whp[:, :], idxs_ap=idxOS_t[:],
                        num_idxs=s_o, num_idxs_reg=s_o,
                        elem_size=P, single_packet=False, queue_num=q1)
                    pairsOD = gpool.tile([P, oc, P], F16, tag="pairsOD")
                    nc.gpsimd.dma_gather(
                        out_ap=pairsOD[:], in_ap=whp[:, :], idxs_ap=idxOD_t[:],
                        num_idxs=s_o, num_idxs_reg=s_o,
                        elem_size=P, single_packet=False, queue_num=q2)

                    # ---------------- dense pipeline ----------------
                    nc.vector.copy_predicated(
                        pairs[:, :, 0:DOUT],
                        parD_v.unsqueeze(2).to_broadcast((P, s_d, DOUT)),
                        pairs[:, :, DOUT:2 * DOUT])
                    whsrc = pairs[:, :, 0:DOUT]
                    u = wpool.tile([P, s_d, DOUT], F16, tag="u")
                    for t in range(GB):
                        stt(u[:, t * j0:(t + 1) * j0, :],
                            whsrc[:, t * j0:(t + 1) * j0, :], ALU.add, 0.0,
                            ALU.add,
                            whd_all[:, bi * GB + t, :].unsqueeze(1)
                            .to_broadcast((P, j0, DOUT)))
                    e32 = wpool.tile([P, s_d, 32], F16, tag="e32")
                    e16 = wpool.tile([P, s_d, 16], F16, tag="e16")
                    ef = wpool.tile([P, s_d], F32, tag="ef")
                    score(u, e32, e16, ef[:], s_d)
                    ex = wpool.tile([P, s_d], F32, tag="ex")
                    nc.scalar.activation(ex[:], ef[:], AF.Exp)
                    exm = wpool.tile([P, s_d], F16, tag="exm")
                    nc.vector.tensor_tensor(exm[:], ex[:], valD_v, op=ALU.mult)
                    v = wpool.tile([P, s_d, DOUT], F16, tag="v")
                    nc.vector.tensor_tensor(
                        v[:], whsrc,
                        exm[:].unsqueeze(2).to_broadcast((P, s_d, DOUT)),
                        op=ALU.mult)
                    # j-tree reduce num: [P, GB, j0, 64] -> [P, GB, 64]
                    tA = wpool.tile([P, GB, half1, DOUT], F16, tag="tA")
                    tB = wpool.tile([P, GB, max(half1 // 2, 1), DOUT], F16,
                                    tag="tB")
                    cur = v[:].rearrange("p (t j) f -> p t j f", t=GB)
                    cn = j0
                    buf = [tA, tB]
                    ti = 0
                    while cn > 1:
                        m = cn // 2
                        nxt = buf[ti][:]
                        ti ^= 1
                        stt(nxt[:, :, 0:m, :], cur[:, :, 0:m, :], ALU.add, 0.0,
                            ALU.add, cur[:, :, m:2 * m, :])
                        if cn % 2:
                            stt(nxt[:, :, 0:1, :], nxt[:, :, 0:1, :], ALU.add,
                                0.0, ALU.add, cur[:, :, 2 * m:2 * m + 1, :])
                        cur, cn = nxt, m
                    numd = cur  # [P, GB, >=1, DOUT], slot 0 holds the sum
                    dend = wpool.tile([P, GB], F32, tag="dend")
                    nc.vector.tensor_reduce(
                        dend[:], exm[:].rearrange("p (t j) -> p t j", t=GB),
                        axis=AX.X, op=ALU.add)

                    # ---------------- overflow pipeline ----------------
                    nc.vector.copy_predicated(
                        pairsOS[:, :, 0:DOUT],
                        parOS_v.unsqueeze(2).to_broadcast((P, oc, DOUT)),
                        pairsOS[:, :, DOUT:2 * DOUT])
                    whsO = pairsOS[:, :, 0:DOUT]
                    nc.vector.copy_predicated(
                        pairsOD[:, :, 0:DOUT],
                        parOD_v.unsqueeze(2).to_broadcast((P, oc, DOUT)),
                        pairsOD[:, :, DOUT:2 * DOUT])
                    uo = wpool.tile([P, oc, DOUT], F16, tag="uo")
                    stt(uo[:], pairsOD[:, :, 0:DOUT], ALU.add, 0.0, ALU.add,
                        whsO)
                    eo32 = wpool.tile([P, oc, 32], F16, tag="eo32")
                    eo16 = wpool.tile([P, oc, 16], F16, tag="eo16")
                    efo = wpool.tile([P, oc], F32, tag="efo")
                    score(uo, eo32, eo16, efo[:], oc)
                    exo = wpool.tile([P, oc], F32, tag="exo")
                    nc.scalar.activation(exo[:], efo[:], AF.Exp)
                    exmo = wpool.tile([P, oc], F16, tag="exmo")
                    nc.vector.tensor_tensor(exmo[:], exo[:], valO_v, op=ALU.mult)
                    vo = wpool.tile([P, oc, DOUT + 1], F16, tag="vo")
                    nc.vector.tensor_tensor(
                        vo[:, :, 0:DOUT], whsO,
                        exmo[:].unsqueeze(2).to_broadcast((P, oc, DOUT)),
                        op=ALU.mult)
                    nc.vector.tensor_copy(vo[:, :, DOUT:DOUT + 1],
                                          exmo[:].unsqueeze(2))
                    oh = wpool.tile([P, oc, P], F16, tag="oh")
                    nc.vector.tensor_tensor(
                        oh[:],
                        dtlO_v.unsqueeze(2).to_broadcast((P, oc, P)),
                        iota_f[:].unsqueeze(1).to_broadcast((P, oc, P)),
                        op=ALU.is_equal)

                    # ---------------- combine + output ----------------
                    obb = npool.tile([P, GB, DOUT], F32, tag="obb")
                    for t in range(GB):
                        ps = apool.tile([P, DOUT + 1], F32, tag="agg")
                        seg_list = segs[t]
                        for k, (c, l0, l1) in enumerate(seg_list):
                            nc.tensor.matmul(
                                ps[:], lhsT=oh[l0:l1, c, :], rhs=vo[l0:l1, c, :],
                                start=(k == 0), stop=(k == len(seg_list) - 1))
                        numf = npool.tile([P, DOUT], F32, tag="numf")
                        stt(numf[:], numd[:, t, 0, :], ALU.add, 0.0, ALU.add,
                            ps[:, 0:DOUT])
                        denf = npool.tile([P, 1], F32, tag="denf")
                        stt(denf[:], dend[:, t:t + 1], ALU.add, 0.0,
                            ALU.add, ps[:, DOUT:DOUT + 1])
                        nc.vector.tensor_scalar_max(denf[:], denf[:], 1e-9)
                        rec = npool.tile([P, 1], F32, tag="rec")
                        nc.vector.reciprocal(rec[:], denf[:])
                        nc.scalar.activation(obb[:, t, :], numf[:], AF.Sigmoid,
                                             scale=rec[:])
                    # permuted batched output write: row = bi*640 + p*GB + t
                    nc.sync.dma_start(
                        out_d[bi * GB * P:(bi + 1) * GB * P, :]
                        .rearrange("(p t) f -> p t f", p=P), obb[:])

    nc.compile()
    return nc


_CACHE = {}


def unpermute_out(arr):
    """Inverse of the permuted batched output write."""
    return arr.reshape(N_BATCH, P, GB, DOUT).transpose(0, 2, 1, 3).reshape(
        N_LOC, DOUT)


def kernel(x, W, b, a, edge_index):
    x = np.ascontiguousarray(np.asarray(x, dtype=np.float32))
    W = np.ascontiguousarray(np.asarray(W, dtype=np.float32))
    b = np.ascontiguousarray(np.asarray(b, dtype=np.float32))
    a = np.ascontiguousarray(np.asarray(a, dtype=np.float32))
    edge_index = np.asarray(edge_index)

    cfg, in_maps, meta = prepare(x, W, b, a, edge_index)
    nc = _CACHE.get(cfg)
    if nc is None:
        nc = build(cfg)
        _CACHE[cfg] = nc

    from concourse.bass_utils import run_bass_kernel_spmd
    res = run_bass_kernel_spmd(nc, in_maps, core_ids=list(range(N_CORES)))
    parts = [unpermute_out(res.results[c]["out"]) for c in range(N_CORES)]
    return np.concatenate(parts, axis=0)[:meta["N"]].astype(np.float32)


# revision 12
# speedup vs baseline: 2.5938x; 1.6648x over previous
"""GATv2 (nn_GATv2_59184649339075) Bass kernel for TRN2, 8-core SPMD.

Self-contained: kernel(**inputs) takes the full unsharded inputs
(x[50000,64], W[64,64], b[64], a[64], edge_index[2,800000] int32) and
returns the full [50000,64] float32 output.

Strategy (dst-partition dense layout + per-tile overflow, no collectives):
  - Host: pad nodes to 51200 (400 tiles of 128); each core owns 50
    consecutive dst tiles. For each dst node, its first J0 incoming edges
    go to a DENSE slot grid [partition = dst%128, j<J0]; the rest go to a
    small per-tile OVERFLOW region of static capacity `cap`.
  - Device per core: Wh = x@W.T + b computed on-chip in f16 (bias folded
    as an augmented contraction row) into a DRAM table stored in a
    partition-major permuted layout so the stage writes are contiguous
    1KB runs; the core's own 56 dst tiles live in SBUF for the whole run.
  - Gathers fetch 256B PAIR rows (two f16 nodes) via SWDGE with int16
    pair indices (table_row>>1 < 32768); copy_predicated in place on the
    gathered pairs selects the parity half.
  - Dense pipeline: s = Whsrc + Whdst (Whdst is a free-dim broadcast of
    the tile's SBUF-resident Wh rows - no gather, no one-hot), LeakyReLU
    on ACT, a-mul + feature tree-reduce for logits (scalar_tensor_tensor
    keeps DVE 2x/4x modes), exp on ACT, validity mask, weighted values,
    then a j tree-reduce for numerator/denominator: aggregation is pure
    free-dim reduction because partition == dst%128.
  - Overflow pipeline (~10% of edges): chunk slots with one-hot is_equal
    and PE matmul accumulation into per-tile PSUM; src AND dst rows both
    gathered as pairs.
  - Output: sigmoid((num_dense + num_ovf) / (den_dense + den_ovf)),
    written batched in a permuted layout, unpermuted on host.
"""
import sys

sys.path.insert(0, "/opt/trn_rl_repo")
from contextlib import ExitStack
from dataclasses import dataclass

import numpy as np

import concourse.bass as bass
import concourse.tile as tile
from concourse import bacc, mybir

F32 = mybir.dt.float32
F16 = mybir.dt.float16
I16 = mybir.dt.int16
I32 = mybir.dt.int32
I8 = mybir.dt.int8
AF = mybir.ActivationFunctionType
ALU = mybir.AluOpType
AX = mybir.AxisListType

N_CORES = 8
P = 128
DIN = 64
DOUT = 64
NSLOPE = 0.2
N_PAD = 51200
TILES = N_PAD // P             # 400
TILES_CORE = TILES // N_CORES  # 50
N_LOC = TILES_CORE * P         # 6400
GB = 5                         # tiles per batch
N_BATCH = TILES_CORE // GB     # 10
GT = 8                         # tiles per wh-stage matmul group
LOC_T = ((TILES_CORE + GT - 1) // GT) * GT  # 56 local tiles (padded)
N_LOC_W = LOC_T * P            # 7168


def row_of_node(n):
    """Table-row permutation: within each 1024-node group the wh stage
    writes partition-major, so node (group, g, lane) lands at row
    group*1024 + lane*8 + g. Works on arrays."""
    return (n // 1024) * 1024 + (n % P) * GT + (n % 1024) // P


@dataclass(frozen=True)
class GatCfg:
    j0: int      # dense slots per dst node
    cap: int     # overflow slots per tile (multiple of 64)

    @property
    def s_d(self):
        return GB * self.j0  # dense chunks (slots/128) per batch

    @property
    def oc(self):
        return (GB * self.cap + P - 1) // P  # overflow chunks per batch

    @property
    def s_o(self):
        return self.oc * P  # padded overflow slots per batch

    def segments(self):
        """Static per-batch overflow layout: for each tile t in the batch,
        the (chunk, lane_lo, lane_hi) pieces covering its cap slots."""
        segs = []
        for t in range(GB):
            lo, hi = t * self.cap, (t + 1) * self.cap
            cur = []
            c = lo // P
            while lo < hi:
                e = min(hi, (c + 1) * P)
                cur.append((c, lo - c * P, e - c * P))
                lo, c = e, c + 1
            segs.append(cur)
        return segs


def wrap16(idx):
    """Slot i of a gather call -> idx array position [i%16, i//16],
    replicated to the 128 partitions."""
    n = len(idx)
    assert n % 16 == 0
    a = idx.reshape(n // 16, 16).T.astype(np.int16)
    return np.tile(a, (8, 1))


def prepare(x, W, b, a, edge_index):
    N = x.shape[0]
    E = edge_index.shape[1]
    assert N <= N_PAD and N_PAD // 2 <= 32768
    src = edge_index[0].astype(np.int64)
    dst = edge_index[1].astype(np.int64)

    order = np.lexsort((src, dst))
    src_s, dst_s = src[order], dst[order]
    deg = np.bincount(dst_s, minlength=N_PAD)
    starts = np.concatenate([[0], np.cumsum(deg)[:-1]])
    rank = np.arange(E) - np.repeat(starts, deg)

    # choose j0 minimizing total gather descriptors (static across cores)
    best = None
    for j0 in range(8, 33, 2):
        ovf_t = np.maximum(deg - j0, 0).reshape(TILES, P).sum(axis=1)
        # cap % 64 == 0 keeps overflow segment base partitions in {0, 64},
        # which the PE matmul base-partition constraint requires
        cap = max(int(np.ceil(ovf_t.max() / 64) * 64), 64)
        s_o = ((GB * cap + P - 1) // P) * P
        desc = TILES_CORE * j0 * P + 2 * N_BATCH * s_o
        if best is None or desc < best[0]:
            best = (desc, j0, cap)
    _, j0, cap = best
    cfg = GatCfg(j0=j0, cap=cap)
    s_d, oc, s_o = cfg.s_d, cfg.oc, cfg.s_o

    tile_id = dst_s >> 7
    b_of = (tile_id % TILES_CORE) // GB
    t_in_b = (tile_id % TILES_CORE) % GB

    dense = rank < j0
    # dense slot within core: (batch*s_d + t_in_b*j0 + rank)*P + dst%128
    d_slot = (b_of * s_d + t_in_b * j0 + rank) * P + (dst_s & (P - 1))
    # overflow: position = cumcount within the dst TILE (edges are sorted
    # by dst so tiles are contiguous among the ovf subset)
    ovf = ~dense
    tile_ovf = tile_id[ovf]
    cnt_t = np.bincount(tile_ovf, minlength=TILES)
    assert cnt_t.max() <= cap
    st_t = np.concatenate([[0], np.cumsum(cnt_t)[:-1]])
    o_pos_tile = np.arange(int(ovf.sum())) - np.repeat(st_t, cnt_t)
    o_slot = (b_of[ovf] * s_o) + t_in_b[ovf] * cap + o_pos_tile

    rsrc = row_of_node(src_s)
    rdst = row_of_node(dst_s)

    xT = np.zeros((DIN + 1, N_PAD), np.float16)
    xT[:DIN, :N] = x.T.astype(np.float16)
    xT[DIN, :] = 1.0
    WT = np.concatenate([W.T.astype(np.float16),
                         b.reshape(1, DOUT).astype(np.float16)])
    a_row = a.reshape(1, DOUT).astype(np.float16)

    core_of = tile_id // TILES_CORE
    core_ovf = core_of[ovf]
    rsrc_o, rdst_o, dst_o = rsrc[ovf], rdst[ovf], dst_s[ovf]

    fd = s_d * P // 16  # idxD cols per batch
    fo = s_o // 16      # idxO cols per batch
    MC = s_d + 2 * oc   # merged f16 mask cols per batch
    PC = s_d + 2 * oc   # merged int8 parity cols per batch

    in_maps = []
    for c in range(N_CORES):
        m_d = dense & (core_of == c)
        m_o = core_ovf == c

        nd = N_BATCH * s_d * P
        idxD = np.zeros(nd, np.int16)
        parD = np.zeros(nd, np.int8)
        valD = np.zeros(nd, np.float16)
        sl = d_slot[m_d]
        idxD[sl] = (rsrc[m_d] >> 1).astype(np.int16)
        parD[sl] = (rsrc[m_d] & 1).astype(np.int8)
        valD[sl] = 1.0

        no = N_BATCH * s_o
        idxOS = np.zeros(no, np.int16)
        idxOD = np.zeros(no, np.int16)
        parOS = np.zeros(no, np.int8)
        parOD = np.zeros(no, np.int8)
        dtlO = np.full(no, -1.0, np.float16)
        valO = np.zeros(no, np.float16)
        so = o_slot[m_o]
        idxOS[so] = (rsrc_o[m_o] >> 1).astype(np.int16)
        idxOD[so] = (rdst_o[m_o] >> 1).astype(np.int16)
        parOS[so] = (rsrc_o[m_o] & 1).astype(np.int8)
        parOD[so] = (rdst_o[m_o] & 1).astype(np.int8)
        dtlO[so] = (dst_o[m_o] & (P - 1)).astype(np.float16)
        valO[so] = 1.0

        idxD_w = np.zeros((P, N_BATCH * fd), np.int16)
        idxOS_w = np.zeros((P, N_BATCH * fo), np.int16)
        idxOD_w = np.zeros((P, N_BATCH * fo), np.int16)
        for i in range(N_BATCH):
            idxD_w[:, i * fd:(i + 1) * fd] = wrap16(
                idxD[i * s_d * P:(i + 1) * s_d * P])
            idxOS_w[:, i * fo:(i + 1) * fo] = wrap16(
                idxOS[i * s_o:(i + 1) * s_o])
            idxOD_w[:, i * fo:(i + 1) * fo] = wrap16(
                idxOD[i * s_o:(i + 1) * s_o])

        def lane_major(v, nslots):
            # v: [nslots*P] slot = chunk*P + lane -> [P(lane), nslots]
            return v.reshape(nslots, P).T

        # merged per-batch planes:
        # f16 msk: [valD s_d | dtlO oc | valO oc]
        # int8 par: [parD s_d | parOS oc | parOD oc]
        msk = np.zeros((P, N_BATCH, MC), np.float16)
        msk[:, :, 0:s_d] = lane_major(valD, N_BATCH * s_d).reshape(P, N_BATCH, s_d)
        msk[:, :, s_d:s_d + oc] = lane_major(dtlO, N_BATCH * oc).reshape(P, N_BATCH, oc)
        msk[:, :, s_d + oc:MC] = lane_major(valO, N_BATCH * oc).reshape(P, N_BATCH, oc)
        par = np.zeros((P, N_BATCH, PC), np.int8)
        par[:, :, 0:s_d] = lane_major(parD, N_BATCH * s_d).reshape(P, N_BATCH, s_d)
        par[:, :, s_d:s_d + oc] = lane_major(parOS, N_BATCH * oc).reshape(P, N_BATCH, oc)
        par[:, :, s_d + oc:PC] = lane_major(parOD, N_BATCH * oc).reshape(P, N_BATCH, oc)

        xTs = np.zeros((DIN + 1, N_LOC_W), np.float16)
        xTs[:, :N_LOC] = xT[:, c * N_LOC:(c + 1) * N_LOC]

        in_maps.append({
            "xT": xT, "xTs": xTs, "WT": WT, "a": a_row,
            "idxD": idxD_w, "idxOS": idxOS_w, "idxOD": idxOD_w,
            "msk": np.ascontiguousarray(msk.reshape(P, N_BATCH * MC)),
            "par": np.ascontiguousarray(par.reshape(P, N_BATCH * PC)),
        })
    return cfg, in_maps, {"N": N}


def build(cfg: GatCfg, reps=1, one_queue=False):
    nc = bacc.Bacc("TRN2", target_bir_lowering=False, debug=False,
                   num_devices=N_CORES, num_swdge_queues=4)
    j0, cap = cfg.j0, cfg.cap
    s_d, oc, s_o = cfg.s_d, cfg.oc, cfg.s_o
    fd = s_d * P // 16
    fo = s_o // 16
    MC = s_d + 2 * oc
    PC = s_d + 2 * oc
    segs = cfg.segments()

    xT_d = nc.dram_tensor("xT", [DIN + 1, N_PAD], F16, kind="ExternalInput").ap()
    xTs_d = nc.dram_tensor("xTs", [DIN + 1, N_LOC_W], F16, kind="ExternalInput").ap()
    WT_d = nc.dram_tensor("WT", [DIN + 1, DOUT], F16, kind="ExternalInput").ap()
    a_d = nc.dram_tensor("a", [1, DOUT], F16, kind="ExternalInput").ap()
    idxD_d = nc.dram_tensor("idxD", [P, N_BATCH * fd], I16, kind="ExternalInput").ap()
    idxOS_d = nc.dram_tensor("idxOS", [P, N_BATCH * fo], I16, kind="ExternalInput").ap()
    idxOD_d = nc.dram_tensor("idxOD", [P, N_BATCH * fo], I16, kind="ExternalInput").ap()
    msk_d = nc.dram_tensor("msk", [P, N_BATCH * MC], F16, kind="ExternalInput").ap()
    par_d = nc.dram_tensor("par", [P, N_BATCH * PC], I8, kind="ExternalInput").ap()
    out_d = nc.dram_tensor("out", [N_LOC, DOUT], F32, kind="ExternalOutput").ap()
    wh_d = nc.dram_tensor("wh", [N_PAD, DOUT], F16).ap()
    # pair view for gathers: row q = table rows (2q, 2q+1), 256B
    whp = wh_d.rearrange("(q two) f -> q (two f)", two=2)

    with tile.TileContext(nc) as tc:
        with ExitStack() as ctx:
            cpool = ctx.enter_context(tc.tile_pool(name="const", bufs=1))
            WT_sb = cpool.tile([DIN + 1, DOUT], F16)
            nc.sync.dma_start(WT_sb[:], WT_d[:, :])
            a_rep = cpool.tile([P, DOUT], F16)
            nc.sync.dma_start(a_rep[:], a_d.to_broadcast((P, DOUT)))
            iota_i = cpool.tile([P, P], I32)
            nc.gpsimd.iota(iota_i[:], pattern=[[1, P]], base=0, channel_multiplier=0)
            iota_f = cpool.tile([P, P], F16)
            nc.vector.tensor_copy(iota_f[:], iota_i[:])
            # the core's own dst-tile Wh rows, resident for the whole run
            whd_all = cpool.tile([P, LOC_T, DOUT], F16)

            # wh table: permuted rows so each partition writes one
            # contiguous 1KB run per group
            with ExitStack() as c2:
                xp = c2.enter_context(tc.tile_pool(name="xt", bufs=3))
                pp = c2.enter_context(tc.tile_pool(name="whps", bufs=4, space="PSUM"))
                wp = c2.enter_context(tc.tile_pool(name="whsb", bufs=3))
                for g in range(TILES // GT):
                    t0 = g * GT
                    xt = xp.tile([DIN + 1, GT * P], F16, tag="xt")
                    nc.sync.dma_start(xt[:], xT_d[:, t0 * P:(t0 + GT) * P])
                    ps = pp.tile([P, GT, DOUT], F32, tag="ps")
                    for j in range(GT):
                        nc.tensor.matmul(ps[:, j, :], lhsT=xt[:, j * P:(j + 1) * P],
                                         rhs=WT_sb[:], start=True, stop=True)
                    whb = wp.tile([P, GT, DOUT], F16, tag="whb")
                    nc.scalar.copy(whb[:], ps[:])
                    nc.sync.dma_start(
                        wh_d[g * GT * P:(g + 1) * GT * P, :]
                        .rearrange("(p g) f -> p g f", p=P), whb[:])
                # local dst tiles -> SBUF-resident whd_all
                for g in range(LOC_T // GT):
                    t0 = g * GT
                    xt = xp.tile([DIN + 1, GT * P], F16, tag="xt")
                    nc.sync.dma_start(xt[:], xTs_d[:, t0 * P:(t0 + GT) * P])
                    ps = pp.tile([P, GT, DOUT], F32, tag="ps")
                    for j in range(GT):
                        nc.tensor.matmul(ps[:, j, :], lhsT=xt[:, j * P:(j + 1) * P],
                                         rhs=WT_sb[:], start=True, stop=True)
                    nc.scalar.copy(whd_all[:, t0:t0 + GT, :], ps[:])

            ipool = ctx.enter_context(tc.tile_pool(name="idx", bufs=2))
            mpool = ctx.enter_context(tc.tile_pool(name="mask", bufs=2))
            gpool = ctx.enter_context(tc.tile_pool(name="gath", bufs=2))
            wpool = ctx.enter_context(tc.tile_pool(name="work", bufs=2))
            apool = ctx.enter_context(tc.tile_pool(name="aggp", bufs=4, space="PSUM"))
            npool = ctx.enter_context(tc.tile_pool(name="outp", bufs=2))

            def stt(out, in0, op0, scalar, op1, in1):
                nc.vector.scalar_tensor_tensor(
                    out, in0, scalar, in1, op0=op0, op1=op1)

            def score(u, e32, e16, ef32, n):
                """u [P,n,64] f16 -> leaky-relu, a-mul (in place),
                tree-reduce features into ef32 [P,n] f32."""
                stt(u[:], u[:], ALU.mult, NSLOPE, ALU.max, u[:])
                stt(u[:], u[:], ALU.add, 0.0, ALU.mult,
                    a_rep[:].unsqueeze(1).to_broadcast((P, n, DOUT)))
                stt(e32[:], u[:, :, 0:32], ALU.add, 0.0, ALU.add, u[:, :, 32:64])
                stt(e16[:], e32[:, :, 0:16], ALU.add, 0.0, ALU.add,
                    e32[:, :, 16:32])
                stt(e32[:, :, 0:8], e16[:, :, 0:8], ALU.add, 0.0, ALU.add,
                    e16[:, :, 8:16])
                stt(e16[:, :, 0:4], e32[:, :, 0:4], ALU.add, 0.0, ALU.add,
                    e32[:, :, 4:8])
                stt(e32[:, :, 0:2], e16[:, :, 0:2], ALU.add, 0.0, ALU.add,
                    e16[:, :, 2:4])
                stt(ef32.unsqueeze(2), e32[:, :, 0:1], ALU.add, 0.0, ALU.add,
                    e32[:, :, 1:2])

            half1 = (j0 + 1) // 2

            for _ in range(reps):
                for bi in range(N_BATCH):
                    idxD_t = ipool.tile([P, fd], I16, tag="id")
                    nc.sync.dma_start(idxD_t[:], idxD_d[:, bi * fd:(bi + 1) * fd])
                    idxOS_t = ipool.tile([P, fo], I16, tag="ios")
                    nc.sync.dma_start(idxOS_t[:], idxOS_d[:, bi * fo:(bi + 1) * fo])
                    idxOD_t = ipool.tile([P, fo], I16, tag="iod")
                    nc.sync.dma_start(idxOD_t[:], idxOD_d[:, bi * fo:(bi + 1) * fo])
                    msk_t = mpool.tile([P, MC], F16, tag="msk")
                    nc.sync.dma_start(msk_t[:], msk_d[:, bi * MC:(bi + 1) * MC])
                    par_t = mpool.tile([P, PC], I8, tag="par")
                    nc.sync.dma_start(par_t[:], par_d[:, bi * PC:(bi + 1) * PC])
                    valD_v = msk_t[:, 0:s_d]
                    dtlO_v = msk_t[:, s_d:s_d + oc]
                    valO_v = msk_t[:, s_d + oc:MC]
                    parD_v = par_t[:, 0:s_d]
                    parOS_v = par_t[:, s_d:s_d + oc]
                    parOD_v = par_t[:, s_d + oc:PC]

                    # balance gather descriptors across the 4 SWDGE
                    # queues: overflow src/dst ride queues 2/3, the dense
                    # call is split so every queue moves ~total/4
                    pairs = gpool.tile([P, s_d, P], F16, tag="pairs")
                    pairsOS = gpool.tile([P, oc, P], F16, tag="pairsOS")
                    pairsOD = gpool.tile([P, oc, P], F16, tag="pairsOD")
                    tot = s_d + 2 * oc
                    tgt = (tot + 3) // 4
                    d01 = min(2 * tgt, s_d)
                    c0 = d01 // 2
                    c1 = d01
                    c2 = c1 + (s_d - d01 + 1) // 2
                    cuts = [0, c0, c1, c2, s_d]
                    for q in range(4):
                        lo, hi = cuts[q], cuts[q + 1]
                        if hi > lo:
                            nc.gpsimd.dma_gather(
                                out_ap=pairs[:, lo:hi, :], in_ap=whp[:, :],
                                idxs_ap=idxD_t[:, lo * 8:hi * 8],
                                num_idxs=(hi - lo) * P,
                                num_idxs_reg=(hi - lo) * P,
                                elem_size=P, single_packet=False,
                                queue_num=0 if one_queue else q)
                    nc.gpsimd.dma_gather(
                        out_ap=pairsOS[:], in_ap=whp[:, :], idxs_ap=idxOS_t[:],
                        num_idxs=s_o, num_idxs_reg=s_o,
                        elem_size=P, single_packet=False,
                        queue_num=0 if one_queue else 2)
                    nc.gpsimd.dma_gather(
                        out_ap=pairsOD[:], in_ap=whp[:, :], idxs_ap=idxOD_t[:],
                        num_idxs=s_o, num_idxs_reg=s_o,
                        elem_size=P, single_packet=False,
                        queue_num=0 if one_queue else 3)

                    # ---------------- dense pipeline ----------------
                    nc.vector.copy_predicated(
                        pairs[:, :, 0:DOUT],
                        parD_v.unsqueeze(2).to_broadcast((P, s_d, DOUT)),
                        pairs[:, :, DOUT:2 * DOUT])
                    whsrc = pairs[:, :, 0:DOUT]
                    u = wpool.tile([P, s_d, DOUT], F16, tag="u")
                    for t in range(GB):
                        stt(u[:, t * j0:(t + 1) * j0, :],
                            whsrc[:, t * j0:(t + 1) * j0, :], ALU.add, 0.0,
                            ALU.add,
                            whd_all[:, bi * GB + t, :].unsqueeze(1)
                            .to_broadcast((P, j0, DOUT)))
                    e32 = wpool.tile([P, s_d, 32], F16, tag="e32")
                    e16 = wpool.tile([P, s_d, 16], F16, tag="e16")
                    ef = wpool.tile([P, s_d], F32, tag="ef")
                    score(u, e32, e16, ef[:], s_d)
                    ex = wpool.tile([P, s_d], F32, tag="ex")
                    nc.scalar.activation(ex[:], ef[:], AF.Exp)
                    exm = wpool.tile([P, s_d], F16, tag="exm")
                    nc.vector.tensor_tensor(exm[:], ex[:], valD_v, op=ALU.mult)
                    v = wpool.tile([P, s_d, DOUT], F16, tag="v")
                    nc.vector.tensor_tensor(
                        v[:], whsrc,
                        exm[:].unsqueeze(2).to_broadcast((P, s_d, DOUT)),
                        op=ALU.mult)
                    # j-tree reduce num: [P, GB, j0, 64] -> [P, GB, 64]
                    tA = wpool.tile([P, GB, half1, DOUT], F16, tag="tA")
                    tB = wpool.tile([P, GB, max(half1 // 2, 1), DOUT], F16,
                                    tag="tB")
                    cur = v[:].rearrange("p (t j) f -> p t j f", t=GB)
                    cn = j0
                    buf = [tA, tB]
                    ti = 0
                    while cn > 1:
                        m = cn // 2
                        nxt = buf[ti][:]
                        ti ^= 1
                        stt(nxt[:, :, 0:m, :], cur[:, :, 0:m, :], ALU.add, 0.0,
                            ALU.add, cur[:, :, m:2 * m, :])
                        if cn % 2:
                            stt(nxt[:, :, 0:1, :], nxt[:, :, 0:1, :], ALU.add,
                                0.0, ALU.add, cur[:, :, 2 * m:2 * m + 1, :])
                        cur, cn = nxt, m
                    numd = cur  # [P, GB, >=1, DOUT], slot 0 holds the sum
                    dend = wpool.tile([P, GB], F32, tag="dend")
                    nc.vector.tensor_reduce(
                        dend[:], exm[:].rearrange("p (t j) -> p t j", t=GB),
                        axis=AX.X, op=ALU.add)

                    # ---------------- overflow pipeline ----------------
                    nc.vector.copy_predicated(
                        pairsOS[:, :, 0:DOUT],
                        parOS_v.unsqueeze(2).to_broadcast((P, oc, DOUT)),
                        pairsOS[:, :, DOUT:2 * DOUT])
                    whsO = pairsOS[:, :, 0:DOUT]
                    nc.vector.copy_predicated(
                        pairsOD[:, :, 0:DOUT],
                        parOD_v.unsqueeze(2).to_broadcast((P, oc, DOUT)),
                        pairsOD[:, :, DOUT:2 * DOUT])
                    uo = wpool.tile([P, oc, DOUT], F16, tag="uo")
                    stt(uo[:], pairsOD[:, :, 0:DOUT], ALU.add, 0.0, ALU.add,
                        whsO)
                    eo32 = wpool.tile([P, oc, 32], F16, tag="eo32")
                    eo16 = wpool.tile([P, oc, 16], F16, tag="eo16")
                    efo = wpool.tile([P, oc], F32, tag="efo")
                    score(uo, eo32, eo16, efo[:], oc)
                    exo = wpool.tile([P, oc], F32, tag="exo")
                    nc.scalar.activation(exo[:], efo[:], AF.Exp)
                    exmo = wpool.tile([P, oc], F16, tag="exmo")
                    nc.vector.tensor_tensor(exmo[:], exo[:], valO_v, op=ALU.mult)
                    vo = wpool.tile([P, oc, DOUT + 1], F16, tag="vo")
                    nc.vector.tensor_tensor(
                        vo[:, :, 0:DOUT], whsO,
                        exmo[:].unsqueeze(2).to_broadcast((P, oc, DOUT)),
                        op=ALU.mult)
                    nc.vector.tensor_copy(vo[:, :, DOUT:DOUT + 1],
                                          exmo[:].unsqueeze(2))
                    oh = wpool.tile([P, oc, P], F16, tag="oh")
                    nc.vector.tensor_tensor(
                        oh[:],
                        dtlO_v.unsqueeze(2).to_broadcast((P, oc, P)),
                        iota_f[:].unsqueeze(1).to_broadcast((P, oc, P)),
                        op=ALU.is_equal)

                    # ---------------- combine + output ----------------
                    obb = npool.tile([P, GB, DOUT], F32, tag="obb")
                    for t in range(GB):
                        ps = apool.tile([P, DOUT + 1], F32, tag="agg")
                        seg_list = segs[t]
                        for k, (c, l0, l1) in enumerate(seg_list):
                            nc.tensor.matmul(
                                ps[:], lhsT=oh[l0:l1, c, :], rhs=vo[l0:l1, c, :],
                                start=(k == 0), stop=(k == len(seg_list) - 1))
                        numf = npool.tile([P, DOUT], F32, tag="numf")
                        stt(numf[:], numd[:, t, 0, :], ALU.add, 0.0, ALU.add,
                            ps[:, 0:DOUT])
                        denf = npool.tile([P, 1], F32, tag="denf")
                        stt(denf[:], dend[:, t:t + 1], ALU.add, 0.0,
                            ALU.add, ps[:, DOUT:DOUT + 1])
                        nc.vector.tensor_scalar_max(denf[:], denf[:], 1e-9)
                        rec = npool.tile([P, 1], F32, tag="rec")
                        nc.vector.reciprocal(rec[:], denf[:])
                        nc.scalar.activation(obb[:, t, :], numf[:], AF.Sigmoid,
                                             scale=rec[:])
                    # permuted batched output write: row = bi*640 + p*GB + t
                    nc.sync.dma_start(
                        out_d[bi * GB * P:(bi + 1) * GB * P, :]
                        .rearrange("(p t) f -> p t f", p=P), obb[:])

    nc.compile()
    return nc


_CACHE = {}


def unpermute_out(arr):
    """Inverse of the permuted batched output write."""
    return arr.reshape(N_BATCH, P, GB, DOUT).transpose(0, 2, 1, 3).reshape(
        N_LOC, DOUT)


def kernel(x, W, b, a, edge_index):
    x = np.ascontiguousarray(np.asarray(x, dtype=np.float32))
    W = np.ascontiguousarray(np.asarray(W, dtype=np.float32))
    b = np.ascontiguousarray(np.asarray(b, dtype=np.float32))
    a = np.ascontiguousarray(np.asarray(a, dtype=np.float32))
    edge_index = np.asarray(edge_index)

    cfg, in_maps, meta = prepare(x, W, b, a, edge_index)
    nc = _CACHE.get(cfg)
    if nc is None:
        nc = build(cfg)
        _CACHE[cfg] = nc

    from concourse.bass_utils import run_bass_kernel_spmd
    res = run_bass_kernel_spmd(nc, in_maps, core_ids=list(range(N_CORES)))
    parts = [unpermute_out(res.results[c]["out"]) for c in range(N_CORES)]
    return np.concatenate(parts, axis=0)[:meta["N"]].astype(np.float32)


# revision 17
# speedup vs baseline: 4.4292x; 1.7076x over previous
"""GATv2 (nn_GATv2_59184649339075) Bass kernel for TRN2, 8-core SPMD.

Self-contained: kernel(**inputs) takes the full unsharded inputs
(x[50000,64], W[64,64], b[64], a[64], edge_index[2,800000] int32) and
returns the full [50000,64] float32 output.

Strategy (dst-partition dense layout + per-tile overflow, no collectives):
  - Host: pad nodes to 51200 (400 tiles of 128); each core owns 50
    consecutive dst tiles. For each dst node, its first J0 incoming edges
    go to a DENSE slot grid [partition = dst%128, j<J0]; the rest go to a
    small per-tile OVERFLOW region of static capacity `cap`.
  - Device per core: Wh = x@W.T + b computed on-chip in f16 (bias folded
    as an augmented contraction row) into a DRAM table stored in a
    partition-major permuted layout so the stage writes are contiguous
    1KB runs; the core's own 56 dst tiles live in SBUF for the whole run.
  - Gathers fetch 256B PAIR rows (two f16 nodes) via SWDGE with int16
    pair indices (table_row>>1 < 32768); copy_predicated in place on the
    gathered pairs selects the parity half.
  - Dense pipeline: s = Whsrc + Whdst (Whdst is a free-dim broadcast of
    the tile's SBUF-resident Wh rows - no gather, no one-hot), LeakyReLU
    on ACT, a-mul + feature tree-reduce for logits (scalar_tensor_tensor
    keeps DVE 2x/4x modes), exp on ACT, validity mask, weighted values,
    then a j tree-reduce for numerator/denominator: aggregation is pure
    free-dim reduction because partition == dst%128.
  - Overflow pipeline (~10% of edges): chunk slots with one-hot is_equal
    and PE matmul accumulation into per-tile PSUM; src AND dst rows both
    gathered as pairs.
  - Output: sigmoid((num_dense + num_ovf) / (den_dense + den_ovf)),
    written batched in a permuted layout, unpermuted on host.
"""
import sys

sys.path.insert(0, "/opt/trn_rl_repo")
from contextlib import ExitStack
from dataclasses import dataclass

import numpy as np

import concourse.bass as bass
import concourse.tile as tile
from concourse import bacc, mybir

F32 = mybir.dt.float32
F16 = mybir.dt.float16
I16 = mybir.dt.int16
I32 = mybir.dt.int32
I8 = mybir.dt.int8
AF = mybir.ActivationFunctionType
ALU = mybir.AluOpType
AX = mybir.AxisListType

N_CORES = 8
P = 128
DIN = 64
DOUT = 64
NSLOPE = 0.2
N_PAD = 51200
TILES = N_PAD // P             # 400
TILES_CORE = TILES // N_CORES  # 50
N_LOC = TILES_CORE * P         # 6400
GB = 5                         # tiles per batch
N_BATCH = TILES_CORE // GB     # 10
GT = 8                         # tiles per wh-stage matmul group
LOC_T = ((TILES_CORE + GT - 1) // GT) * GT  # 56 local tiles (padded)
N_LOC_W = LOC_T * P            # 7168


def row_of_node(n):
    """Table-row permutation: within each 1024-node group the wh stage
    writes partition-major, so node (group, g, lane) lands at row
    group*1024 + lane*8 + g. Works on arrays."""
    return (n // 1024) * 1024 + (n % P) * GT + (n % 1024) // P


@dataclass(frozen=True)
class GatCfg:
    j0: int      # dense slots per dst node
    cap: int     # overflow slots per tile (multiple of 64)

    @property
    def s_d(self):
        return GB * self.j0  # dense chunks (slots/128) per batch

    @property
    def oc(self):
        return (GB * self.cap + P - 1) // P  # overflow chunks per batch

    @property
    def s_o(self):
        return self.oc * P  # padded overflow slots per batch

    def segments(self):
        """Static per-batch overflow layout: for each tile t in the batch,
        the (chunk, lane_lo, lane_hi) pieces covering its cap slots."""
        segs = []
        for t in range(GB):
            lo = t * self.cap
            hi = (t + 1) * self.cap if t < GB - 1 else self.s_o
            cur = []
            c = lo // P
            while lo < hi:
                e = min(hi, (c + 1) * P)
                cur.append((c, lo - c * P, e - c * P))
                lo, c = e, c + 1
            segs.append(cur)
        return segs


def wrap16(idx):
    """Slot i of a gather call -> idx array position [i%16, i//16],
    replicated to the 128 partitions."""
    n = len(idx)
    assert n % 16 == 0
    a = idx.reshape(n // 16, 16).T.astype(np.int16)
    return np.tile(a, (8, 1))


def prepare(x, W, b, a, edge_index):
    N = x.shape[0]
    E = edge_index.shape[1]
    assert N <= N_PAD and N_PAD // 2 <= 32768
    src = edge_index[0].astype(np.int64)
    dst = edge_index[1].astype(np.int64)

    order = np.lexsort((src, dst))
    src_s, dst_s = src[order], dst[order]
    deg = np.bincount(dst_s, minlength=N_PAD)
    starts = np.concatenate([[0], np.cumsum(deg)[:-1]])
    rank = np.arange(E) - np.repeat(starts, deg)

    # choose j0 minimizing total gather descriptors (static across cores)
    best = None
    for j0 in range(8, 33, 2):
        ovf_t = np.maximum(deg - j0, 0).reshape(TILES, P).sum(axis=1)
        # cap % 64 == 0 keeps overflow segment base partitions in {0, 64},
        # which the PE matmul base-partition constraint requires
        cap = max(int(np.ceil(ovf_t.max() / 64) * 64), 64)
        s_o = ((GB * cap + P - 1) // P) * P
        # overflow slots cost ~2x a dense slot (gather + one-hot matmul
        # machinery on DVE/PE/ACT), so weight them double when sizing
        cost = TILES_CORE * j0 * P + 2 * N_BATCH * s_o
        if best is None or cost < best[0]:
            best = (cost, j0, cap)
    _, j0, cap = best
    cfg = GatCfg(j0=j0, cap=cap)
    s_d, oc, s_o = cfg.s_d, cfg.oc, cfg.s_o

    tile_id = dst_s >> 7
    b_of = (tile_id % TILES_CORE) // GB
    t_in_b = (tile_id % TILES_CORE) % GB

    dense = rank < j0
    # dense slot within core: (batch*s_d + t_in_b*j0 + rank)*P + dst%128
    d_slot = (b_of * s_d + t_in_b * j0 + rank) * P + (dst_s & (P - 1))
    # overflow: position = cumcount within the dst TILE (edges are sorted
    # by dst so tiles are contiguous among the ovf subset)
    ovf = ~dense
    tile_ovf = tile_id[ovf]
    cnt_t = np.bincount(tile_ovf, minlength=TILES)
    assert cnt_t.max() <= cap
    st_t = np.concatenate([[0], np.cumsum(cnt_t)[:-1]])
    o_pos_tile = np.arange(int(ovf.sum())) - np.repeat(st_t, cnt_t)
    o_slot = (b_of[ovf] * s_o) + t_in_b[ovf] * cap + o_pos_tile

    rsrc = row_of_node(src_s)
    rdst = row_of_node(dst_s)

    xT = np.zeros((DIN + 1, N_PAD), np.float16)
    xT[:DIN, :N] = x.T.astype(np.float16)
    xT[DIN, :] = 1.0
    WT = np.concatenate([W.T.astype(np.float16),
                         b.reshape(1, DOUT).astype(np.float16)])
    a_row = a.reshape(1, DOUT).astype(np.float16)

    core_of = tile_id // TILES_CORE
    core_ovf = core_of[ovf]
    rsrc_o, rdst_o, dst_o = rsrc[ovf], rdst[ovf], dst_s[ovf]

    fd = s_d * P // 16  # idxD cols per batch
    fo = s_o // 16      # idxO cols per batch
    MC = s_d + 2 * oc   # merged f16 mask cols per batch
    PC = s_d + oc       # merged int8 parity cols per batch

    in_maps = []
    for c in range(N_CORES):
        m_d = dense & (core_of == c)
        m_o = core_ovf == c

        nd = N_BATCH * s_d * P
        idxD = np.zeros(nd, np.int16)
        parD = np.zeros(nd, np.int8)
        valD = np.zeros(nd, np.float16)
        sl = d_slot[m_d]
        idxD[sl] = (rsrc[m_d] >> 1).astype(np.int16)
        parD[sl] = (rsrc[m_d] & 1).astype(np.int8)
        valD[sl] = 1.0

        no = N_BATCH * s_o
        idxOS = np.zeros(no, np.int16)
        parOS = np.zeros(no, np.int8)
        dtlO = np.full(no, -1.0, np.float16)
        valO = np.zeros(no, np.float16)
        so = o_slot[m_o]
        idxOS[so] = (rsrc_o[m_o] >> 1).astype(np.int16)
        parOS[so] = (rsrc_o[m_o] & 1).astype(np.int8)
        dtlO[so] = (dst_o[m_o] & (P - 1)).astype(np.float16)
        valO[so] = 1.0

        idxD_w = np.zeros((P, N_BATCH * fd), np.int16)
        idxOS_w = np.zeros((P, N_BATCH * fo), np.int16)
        for i in range(N_BATCH):
            idxD_w[:, i * fd:(i + 1) * fd] = wrap16(
                idxD[i * s_d * P:(i + 1) * s_d * P])
            idxOS_w[:, i * fo:(i + 1) * fo] = wrap16(
                idxOS[i * s_o:(i + 1) * s_o])

        def lane_major(v, nslots):
            # v: [nslots*P] slot = chunk*P + lane -> [P(lane), nslots]
            return v.reshape(nslots, P).T

        # merged per-batch planes:
        # f16 msk: [valD s_d | dtlO oc | valO oc]
        # int8 par: [parD s_d | parOS oc | parOD oc]
        msk = np.zeros((P, N_BATCH, MC), np.float16)
        msk[:, :, 0:s_d] = lane_major(valD, N_BATCH * s_d).reshape(P, N_BATCH, s_d)
        msk[:, :, s_d:s_d + oc] = lane_major(dtlO, N_BATCH * oc).reshape(P, N_BATCH, oc)
        msk[:, :, s_d + oc:MC] = lane_major(valO, N_BATCH * oc).reshape(P, N_BATCH, oc)
        par = np.zeros((P, N_BATCH, PC), np.int8)
        par[:, :, 0:s_d] = lane_major(parD, N_BATCH * s_d).reshape(P, N_BATCH, s_d)
        par[:, :, s_d:PC] = lane_major(parOS, N_BATCH * oc).reshape(P, N_BATCH, oc)

        xTs = np.zeros((DIN + 1, N_LOC_W), np.float16)
        xTs[:, :N_LOC] = xT[:, c * N_LOC:(c + 1) * N_LOC]

        in_maps.append({
            "xT": xT, "xTs": xTs, "WT": WT, "a": a_row,
            "idxD": idxD_w, "idxOS": idxOS_w,
            "msk": np.ascontiguousarray(msk.reshape(P, N_BATCH * MC)),
            "par": np.ascontiguousarray(par.reshape(P, N_BATCH * PC)),
        })
    return cfg, in_maps, {"N": N}


def build(cfg: GatCfg, reps=1, one_queue=False):
    nc = bacc.Bacc("TRN2", target_bir_lowering=False, debug=False,
                   num_devices=N_CORES, num_swdge_queues=4)
    j0, cap = cfg.j0, cfg.cap
    s_d, oc, s_o = cfg.s_d, cfg.oc, cfg.s_o
    fd = s_d * P // 16
    fo = s_o // 16
    MC = s_d + 2 * oc
    PC = s_d + oc
    segs = cfg.segments()
    ch_segs = [[] for _ in range(oc)]
    for t in range(GB):
        for (c, l0, l1) in segs[t]:
            ch_segs[c].append((t, l0, l1))

    xT_d = nc.dram_tensor("xT", [DIN + 1, N_PAD], F16, kind="ExternalInput").ap()
    xTs_d = nc.dram_tensor("xTs", [DIN + 1, N_LOC_W], F16, kind="ExternalInput").ap()
    WT_d = nc.dram_tensor("WT", [DIN + 1, DOUT], F16, kind="ExternalInput").ap()
    a_d = nc.dram_tensor("a", [1, DOUT], F16, kind="ExternalInput").ap()
    idxD_d = nc.dram_tensor("idxD", [P, N_BATCH * fd], I16, kind="ExternalInput").ap()
    idxOS_d = nc.dram_tensor("idxOS", [P, N_BATCH * fo], I16, kind="ExternalInput").ap()
    msk_d = nc.dram_tensor("msk", [P, N_BATCH * MC], F16, kind="ExternalInput").ap()
    par_d = nc.dram_tensor("par", [P, N_BATCH * PC], I8, kind="ExternalInput").ap()
    out_d = nc.dram_tensor("out", [N_LOC, DOUT], F32, kind="ExternalOutput").ap()
    wh_d = nc.dram_tensor("wh", [N_PAD, DOUT], F16).ap()
    # pair view for gathers: row q = table rows (2q, 2q+1), 256B
    whp = wh_d.rearrange("(q two) f -> q (two f)", two=2)

    with tile.TileContext(nc) as tc:
        with ExitStack() as ctx:
            cpool = ctx.enter_context(tc.tile_pool(name="const", bufs=1))
            WT_sb = cpool.tile([DIN + 1, DOUT], F16)
            nc.sync.dma_start(WT_sb[:], WT_d[:, :])
            a_rep = cpool.tile([P, DOUT], F16)
            nc.sync.dma_start(a_rep[:], a_d.to_broadcast((P, DOUT)))
            iota_i = cpool.tile([P, P], I32)
            nc.gpsimd.iota(iota_i[:], pattern=[[1, P]], base=0, channel_multiplier=0)
            iota_f = cpool.tile([P, P], F16)
            nc.vector.tensor_copy(iota_f[:], iota_i[:])
            iota_p = cpool.tile([P, 1], I32)
            nc.gpsimd.iota(iota_p[:], pattern=[[1, 1]], base=0,
                           channel_multiplier=1)
            iota_pf = cpool.tile([P, 1], F16)
            nc.vector.tensor_copy(iota_pf[:], iota_p[:])
            ident = cpool.tile([P, P], F16)
            nc.vector.tensor_tensor(
                ident[:], iota_f[:],
                iota_pf[:].to_broadcast((P, P)), op=ALU.is_equal)
            # the core's own dst-tile Wh rows, resident for the whole run
            whd_all = cpool.tile([P, LOC_T, DOUT], F16)

            # wh table: permuted rows so each partition writes one
            # contiguous 1KB run per group
            with ExitStack() as c2:
                xp = c2.enter_context(tc.tile_pool(name="xt", bufs=3))
                pp = c2.enter_context(tc.tile_pool(name="whps", bufs=4, space="PSUM"))
                wp = c2.enter_context(tc.tile_pool(name="whsb", bufs=3))
                for g in range(TILES // GT):
                    t0 = g * GT
                    xt = xp.tile([DIN + 1, GT * P], F16, tag="xt")
                    nc.sync.dma_start(xt[:], xT_d[:, t0 * P:(t0 + GT) * P])
                    ps = pp.tile([P, GT, DOUT], F32, tag="ps")
                    for j in range(GT):
                        nc.tensor.matmul(ps[:, j, :], lhsT=xt[:, j * P:(j + 1) * P],
                                         rhs=WT_sb[:], start=True, stop=True)
                    whb = wp.tile([P, GT, DOUT], F16, tag="whb")
                    nc.scalar.copy(whb[:], ps[:])
                    nc.sync.dma_start(
                        wh_d[g * GT * P:(g + 1) * GT * P, :]
                        .rearrange("(p g) f -> p g f", p=P), whb[:])
                # local dst tiles -> SBUF-resident whd_all
                for g in range(LOC_T // GT):
                    t0 = g * GT
                    xt = xp.tile([DIN + 1, GT * P], F16, tag="xt")
                    nc.sync.dma_start(xt[:], xTs_d[:, t0 * P:(t0 + GT) * P])
                    ps = pp.tile([P, GT, DOUT], F32, tag="ps")
                    for j in range(GT):
                        nc.tensor.matmul(ps[:, j, :], lhsT=xt[:, j * P:(j + 1) * P],
                                         rhs=WT_sb[:], start=True, stop=True)
                    nc.scalar.copy(whd_all[:, t0:t0 + GT, :], ps[:])

            ipool = ctx.enter_context(tc.tile_pool(name="idx", bufs=2))
            mpool = ctx.enter_context(tc.tile_pool(name="mask", bufs=2))
            gpool = ctx.enter_context(tc.tile_pool(name="gath", bufs=2))
            wpool = ctx.enter_context(tc.tile_pool(name="work", bufs=2))
            apool = ctx.enter_context(tc.tile_pool(name="aggp", bufs=2, space="PSUM"))
            npool = ctx.enter_context(tc.tile_pool(name="outp", bufs=2))

            def stt(out, in0, op0, scalar, op1, in1):
                nc.vector.scalar_tensor_tensor(
                    out, in0, scalar, in1, op0=op0, op1=op1)

            def score(u, e32, e16, ef32, n):
                """u [P,n,64] f16 -> leaky-relu, a-mul (in place),
                tree-reduce features into ef32 [P,n] f32."""
                stt(u[:], u[:], ALU.mult, NSLOPE, ALU.max, u[:])
                stt(u[:], u[:], ALU.add, 0.0, ALU.mult,
                    a_rep[:].unsqueeze(1).to_broadcast((P, n, DOUT)))
                stt(e32[:], u[:, :, 0:32], ALU.add, 0.0, ALU.add, u[:, :, 32:64])
                stt(e16[:], e32[:, :, 0:16], ALU.add, 0.0, ALU.add,
                    e32[:, :, 16:32])
                stt(e32[:, :, 0:8], e16[:, :, 0:8], ALU.add, 0.0, ALU.add,
                    e16[:, :, 8:16])
                stt(e16[:, :, 0:4], e32[:, :, 0:4], ALU.add, 0.0, ALU.add,
                    e32[:, :, 4:8])
                stt(e32[:, :, 0:2], e16[:, :, 0:2], ALU.add, 0.0, ALU.add,
                    e16[:, :, 2:4])
                stt(ef32.unsqueeze(2), e32[:, :, 0:1], ALU.add, 0.0, ALU.add,
                    e32[:, :, 1:2])

            half1 = (j0 + 1) // 2

            for _ in range(reps):
                for bi in range(N_BATCH):
                    idxD_t = ipool.tile([P, fd], I16, tag="id")
                    nc.sync.dma_start(idxD_t[:], idxD_d[:, bi * fd:(bi + 1) * fd])
                    idxOS_t = ipool.tile([P, fo], I16, tag="ios")
                    nc.sync.dma_start(idxOS_t[:], idxOS_d[:, bi * fo:(bi + 1) * fo])
                    msk_t = mpool.tile([P, MC], F16, tag="msk")
                    nc.sync.dma_start(msk_t[:], msk_d[:, bi * MC:(bi + 1) * MC])
                    par_t = mpool.tile([P, PC], I8, tag="par")
                    nc.sync.dma_start(par_t[:], par_d[:, bi * PC:(bi + 1) * PC])
                    valD_v = msk_t[:, 0:s_d]
                    dtlO_v = msk_t[:, s_d:s_d + oc]
                    valO_v = msk_t[:, s_d + oc:MC]
                    parD_v = par_t[:, 0:s_d]
                    parOS_v = par_t[:, s_d:PC]

                    # balance gather descriptors across the 4 SWDGE
                    # queues: overflow src/dst ride queues 2/3, the dense
                    # call is split so every queue moves ~total/4
                    pairs = gpool.tile([P, s_d, P], F16, tag="pairs")
                    pairsOS = gpool.tile([P, oc, P], F16, tag="pairsOS")
                    tot = s_d + oc
                    tgt = (tot + 3) // 4
                    d3 = max(tgt - oc, 0)
                    rem = s_d - d3
                    c0 = (rem + 2) // 3
                    c1 = min(2 * c0, rem)
                    cuts = [0, c0, c1, rem, s_d]
                    for q in range(4):
                        lo, hi = cuts[q], cuts[q + 1]
                        if hi > lo:
                            nc.gpsimd.dma_gather(
                                out_ap=pairs[:, lo:hi, :], in_ap=whp[:, :],
                                idxs_ap=idxD_t[:, lo * 8:hi * 8],
                                num_idxs=(hi - lo) * P,
                                num_idxs_reg=(hi - lo) * P,
                                elem_size=P, single_packet=False,
                                queue_num=0 if one_queue else q)
                    nc.gpsimd.dma_gather(
                        out_ap=pairsOS[:], in_ap=whp[:, :], idxs_ap=idxOS_t[:],
                        num_idxs=s_o, num_idxs_reg=s_o,
                        elem_size=P, single_packet=False,
                        queue_num=0 if one_queue else 3)

                    # ---------------- dense pipeline ----------------
                    nc.vector.copy_predicated(
                        pairs[:, :, 0:DOUT],
                        parD_v.unsqueeze(2).to_broadcast((P, s_d, DOUT)),
                        pairs[:, :, DOUT:2 * DOUT])
                    whsrc = pairs[:, :, 0:DOUT]
                    u = wpool.tile([P, s_d, DOUT], F16, tag="u")
                    for t in range(GB):
                        stt(u[:, t * j0:(t + 1) * j0, :],
                            whsrc[:, t * j0:(t + 1) * j0, :], ALU.add, 0.0,
                            ALU.add,
                            whd_all[:, bi * GB + t, :].unsqueeze(1)
                            .to_broadcast((P, j0, DOUT)))
                    e32 = wpool.tile([P, s_d, 32], F16, tag="e32")
                    e16 = wpool.tile([P, s_d, 16], F16, tag="e16")
                    ef = wpool.tile([P, s_d], F32, tag="ef")
                    score(u, e32, e16, ef[:], s_d)
                    ex = wpool.tile([P, s_d], F32, tag="ex")
                    nc.scalar.activation(ex[:], ef[:], AF.Exp)
                    exm = wpool.tile([P, s_d], F16, tag="exm")
                    nc.vector.tensor_tensor(exm[:], ex[:], valD_v, op=ALU.mult)
                    v = wpool.tile([P, s_d, DOUT], F16, tag="v")
                    nc.vector.tensor_tensor(
                        v[:], whsrc,
                        exm[:].unsqueeze(2).to_broadcast((P, s_d, DOUT)),
                        op=ALU.mult)
                    # j-tree reduce num: [P, GB, j0, 64] -> [P, GB, 64]
                    tA = wpool.tile([P, GB, half1, DOUT], F16, tag="tA")
                    tB = wpool.tile([P, GB, max(half1 // 2, 1), DOUT], F16,
                                    tag="tB")
                    cur = v[:].rearrange("p (t j) f -> p t j f", t=GB)
                    cn = j0
                    buf = [tA, tB]
                    ti = 0
                    while cn > 1:
                        m = cn // 2
                        nxt = buf[ti][:]
                        ti ^= 1
                        stt(nxt[:, :, 0:m, :], cur[:, :, 0:m, :], ALU.add, 0.0,
                            ALU.add, cur[:, :, m:2 * m, :])
                        if cn % 2:
                            stt(nxt[:, :, 0:1, :], nxt[:, :, 0:1, :], ALU.add,
                                0.0, ALU.add, cur[:, :, 2 * m:2 * m + 1, :])
                        cur, cn = nxt, m
                    numd = cur  # [P, GB, >=1, DOUT], slot 0 holds the sum
                    dend = wpool.tile([P, GB], F32, tag="dend")
                    nc.vector.tensor_reduce(
                        dend[:], exm[:].rearrange("p (t j) -> p t j", t=GB),
                        axis=AX.X, op=ALU.add)

                    # ---------------- overflow pipeline ----------------
                    nc.vector.copy_predicated(
                        pairsOS[:, :, 0:DOUT],
                        parOS_v.unsqueeze(2).to_broadcast((P, oc, DOUT)),
                        pairsOS[:, :, DOUT:2 * DOUT])
                    whsO = pairsOS[:, :, 0:DOUT]
                    # one-hot first (also consumed by whdst reconstruction)
                    oh = wpool.tile([P, oc, P], F16, tag="oh")
                    nc.vector.tensor_tensor(
                        oh[:],
                        dtlO_v.unsqueeze(2).to_broadcast((P, oc, P)),
                        iota_f[:].unsqueeze(1).to_broadcast((P, oc, P)),
                        op=ALU.is_equal)
                    # whdst[slot] = sum_n ohT[n, slot] * whd_all[n, tile(slot)]
                    whdO = wpool.tile([P, oc, DOUT], F16, tag="whdO")
                    for c in range(oc):
                        pst = apool.tile([P, P], F16, tag="pst")
                        nc.tensor.transpose(pst[:], oh[:, c, :], ident[:])
                        ohT = wpool.tile([P, P], F16, tag="ohT")
                        nc.scalar.copy(ohT[:], pst[:])
                        for (t, l0, l1) in ch_segs[c]:
                            # full-lane matmul against this tile's Wh rows;
                            # only the segment's lanes are kept (engines
                            # cannot shift partitions, so out stays at
                            # tile_position (0, 0))
                            psd = apool.tile([P, DOUT], F32, tag="psd")
                            nc.tensor.matmul(
                                psd[:], lhsT=ohT[:],
                                rhs=whd_all[:, bi * GB + t, :],
                                start=True, stop=True)
                            nc.scalar.copy(whdO[l0:l1, c, :], psd[l0:l1, :])
                    uo = wpool.tile([P, oc, DOUT], F16, tag="uo")
                    stt(uo[:], whdO[:], ALU.add, 0.0, ALU.add, whsO)
                    eo32 = wpool.tile([P, oc, 32], F16, tag="eo32")
                    eo16 = wpool.tile([P, oc, 16], F16, tag="eo16")
                    efo = wpool.tile([P, oc], F32, tag="efo")
                    score(uo, eo32, eo16, efo[:], oc)
                    exo = wpool.tile([P, oc], F32, tag="exo")
                    nc.scalar.activation(exo[:], efo[:], AF.Exp)
                    exmo = wpool.tile([P, oc], F16, tag="exmo")
                    nc.vector.tensor_tensor(exmo[:], exo[:], valO_v, op=ALU.mult)
                    vo = wpool.tile([P, oc, DOUT + 1], F16, tag="vo")
                    nc.vector.tensor_tensor(
                        vo[:, :, 0:DOUT], whsO,
                        exmo[:].unsqueeze(2).to_broadcast((P, oc, DOUT)),
                        op=ALU.mult)
                    nc.vector.tensor_copy(vo[:, :, DOUT:DOUT + 1],
                                          exmo[:].unsqueeze(2))

                    # ---------------- combine + output ----------------
                    obb = npool.tile([P, GB, DOUT], F32, tag="obb")
                    for t in range(GB):
                        ps = apool.tile([P, DOUT + 1], F32, tag="agg")
                        seg_list = segs[t]
                        for k, (c, l0, l1) in enumerate(seg_list):
                            nc.tensor.matmul(
                                ps[:], lhsT=oh[l0:l1, c, :], rhs=vo[l0:l1, c, :],
                                start=(k == 0), stop=(k == len(seg_list) - 1))
                        numf = npool.tile([P, DOUT], F32, tag="numf")
                        stt(numf[:], numd[:, t, 0, :], ALU.add, 0.0, ALU.add,
                            ps[:, 0:DOUT])
                        denf = npool.tile([P, 1], F32, tag="denf")
                        stt(denf[:], dend[:, t:t + 1], ALU.add, 0.0,
                            ALU.add, ps[:, DOUT:DOUT + 1])
                        nc.vector.tensor_scalar_max(denf[:], denf[:], 1e-9)
                        rec = npool.tile([P, 1], F32, tag="rec")
                        nc.vector.reciprocal(rec[:], denf[:])
                        nc.scalar.activation(obb[:, t, :], numf[:], AF.Sigmoid,
                                             scale=rec[:])
                    # permuted batched output write: row = bi*640 + p*GB + t
                    nc.sync.dma_start(
                        out_d[bi * GB * P:(bi + 1) * GB * P, :]
                        .rearrange("(p t) f -> p t f", p=P), obb[:])

    nc.compile()
    return nc


_CACHE = {}


def unpermute_out(arr):
    """Inverse of the permuted batched output write."""
    return arr.reshape(N_BATCH, P, GB, DOUT).transpose(0, 2, 1, 3).reshape(
        N_LOC, DOUT)


def kernel(x, W, b, a, edge_index):
    x = np.ascontiguousarray(np.asarray(x, dtype=np.float32))
    W = np.ascontiguousarray(np.asarray(W, dtype=np.float32))
    b = np.ascontiguousarray(np.asarray(b, dtype=np.float32))
    a = np.ascontiguousarray(np.asarray(a, dtype=np.float32))
    edge_index = np.asarray(edge_index)

    cfg, in_maps, meta = prepare(x, W, b, a, edge_index)
    nc = _CACHE.get(cfg)
    if nc is None:
        nc = build(cfg)
        _CACHE[cfg] = nc

    from concourse.bass_utils import run_bass_kernel_spmd
    res = run_bass_kernel_spmd(nc, in_maps, core_ids=list(range(N_CORES)))
    parts = [unpermute_out(res.results[c]["out"]) for c in range(N_CORES)]
    return np.concatenate(parts, axis=0)[:meta["N"]].astype(np.float32)
